# revision 13
# baseline (speedup 1.0000x reference)
"""Bass/Tile TRN2 kernel for a non-local attention block (BaseNonLocalBlock).

Contract: kernel(**inputs) takes the FULL inputs of the nn.Module problem
(B=1, D=256, H=4, N=4096) and returns the FULL output [1, 256, 4096].

Sharding: query columns of the N x N attention are split across the 8
NeuronCores (512 queries per core). K/V projections are computed
redundantly on every core (cheap); each core produces its own output
column slice and the host concatenates.

Per-core algorithm (flash-attention style, scores never hit HBM):
  Q = (Wq/8) @ xq                      [256, 512]  (1/sqrt(DH) folded in)
  per 512-key block ib (projected JUST-IN-TIME, interleaved into the
  attention loop two pairs ahead):
    K[:, ib]  = Wk @ x[:, ib]          -> k_sb[128, 2, 4096] (co-major)
    V_T[ib]   = x[:, ib]^T @ Wv^T      -> vt[128, it, h, 68] (ones col 64)
  attention loop over PAIRS of 128-key chunks (it = 2p, 2p+1):
    S_T = K_h^T @ Q_h                  [128, 2, 512] PSUM per (it, hp)
    el  = spt * S_T                    DVE mult (4 per pair)
    e2  = exp(el)                      ONE ACT exp, FD=4096 (batched pair)
    msgs for pair p-1: mps[h] += V_T^T @ e2   (PSUM accum; row 64 = denom)
  tail: per-head denom gather -> one DVE reciprocal -> gpsimd broadcast
  -> normalize; out = xq + W3 @ relu(bn2(W2 @ relu(bn1(W1 @ msg))))

Matmul operands are bf16; accumulation fp32 in PSUM; residual reads a
separate fp32 copy of x so the dominant term is exact.
"""

import numpy as np
from contextlib import ExitStack

D = 256
N = 4096
NQ = 512          # queries per core
H = 4
DH = 64
NCORES = 8
NIT = N // 128    # 32 key chunks
NPAIR = NIT // 2  # 16 chunk pairs
VTS = 68          # padded per-head stride in the V_T-aug tile

_CACHE = {}


def _build(has_bq, has_bk, has_bv, has_b3, has_b12):
    import concourse.bass as bass
    import concourse.tile as tile
    from concourse import bacc, mybir

    F32 = mybir.dt.float32
    BF16 = mybir.dt.bfloat16
    Id = mybir.ActivationFunctionType.Identity
    Exp = mybir.ActivationFunctionType.Exp
    Relu = mybir.ActivationFunctionType.Relu

    nc = bacc.Bacc("TRN2", target_bir_lowering=False, debug=False,
                   num_devices=NCORES)

    # DRAM I/O (per core)
    x_d = nc.dram_tensor("x", [D, N], BF16, kind="ExternalInput").ap()
    xq_d = nc.dram_tensor("xq", [D, NQ], BF16, kind="ExternalInput").ap()
    xqr_d = nc.dram_tensor("xqr", [D, NQ], F32, kind="ExternalInput").ap()
    spt_d = nc.dram_tensor("spt", [N, NQ], BF16, kind="ExternalInput").ap()
    wqt_d = nc.dram_tensor("wqt", [D, D], BF16, kind="ExternalInput").ap()
    wkt_d = nc.dram_tensor("wkt", [D, D], BF16, kind="ExternalInput").ap()
    wvt_d = nc.dram_tensor("wvt", [D, D], BF16, kind="ExternalInput").ap()
    w1t_d = nc.dram_tensor("w1t", [D, 128], BF16, kind="ExternalInput").ap()
    w2t_d = nc.dram_tensor("w2t", [128, 128], BF16, kind="ExternalInput").ap()
    w3t_d = nc.dram_tensor("w3t", [128, D], BF16, kind="ExternalInput").ap()
    bq_d = nc.dram_tensor("bq2", [128, 2], F32, kind="ExternalInput").ap()
    bk_d = nc.dram_tensor("bk2", [128, 2], F32, kind="ExternalInput").ap()
    bv_d = nc.dram_tensor("bv2", [128, 2], F32, kind="ExternalInput").ap()
    b1_d = nc.dram_tensor("b1f", [128, 1], F32, kind="ExternalInput").ap()
    b2_d = nc.dram_tensor("b2f", [128, 1], F32, kind="ExternalInput").ap()
    b3_d = nc.dram_tensor("b32", [128, 2], F32, kind="ExternalInput").ap()
    out_d = nc.dram_tensor("out", [D, NQ], F32, kind="ExternalOutput").ap()

    # pair-granular view of spt: pair p, partition, (u, queries)
    spt_t4 = spt_d.rearrange("(t u p) o -> t p u o", u=2, p=128)
    # (ci, p) views of the [256, .] weight matrices
    wq3_d = wqt_d.rearrange("(c p) o -> p c o", p=128)
    wk3_d = wkt_d.rearrange("(c p) o -> p c o", p=128)
    wv3_d = wvt_d.rearrange("(c p) o -> p c o", p=128)
    w13_d = w1t_d.rearrange("(c p) o -> p c o", p=128)
    xq3_d = xq_d.rearrange("(c p) o -> p c o", p=128)
    xqr3_d = xqr_d.rearrange("(c p) o -> p c o", p=128)

    with tile.TileContext(nc) as tc, ExitStack() as ctx:
        sb = ctx.enter_context(tc.tile_pool(name="sb", bufs=1))
        spt_pool = ctx.enter_context(tc.tile_pool(name="sptp", bufs=4))
        el_pool = ctx.enter_context(tc.tile_pool(name="elp", bufs=2))
        e2_pool = ctx.enter_context(tc.tile_pool(name="e2p", bufs=2))
        ps = ctx.enter_context(tc.tile_pool(name="ps", bufs=2, space="PSUM"))
        ps_m = ctx.enter_context(tc.tile_pool(name="psm", bufs=1, space="PSUM"))

        # ---- ACT exp-table preload (overlaps the DMA ramp) ----
        warm = sb.tile([1, 2], F32, name="warm")
        nc.vector.memset(warm[:], 0.0)
        nc.scalar.activation(warm[:], warm[:], Exp)

        # ---- weights + inputs: big DMAs, earliest-needed first ----
        wqt = sb.tile([128, 2, D], BF16, name="wqt")
        wkt = sb.tile([128, 2, D], BF16, name="wkt")
        wvt = sb.tile([128, 2, D], BF16, name="wvt")
        xq = sb.tile([128, 2, NQ], BF16, name="xq")
        nc.sync.dma_start(wqt[:], wq3_d[:])
        nc.sync.dma_start(xq[:], xq3_d[:])
        nc.sync.dma_start(wkt[:], wk3_d[:])
        nc.sync.dma_start(wvt[:], wv3_d[:])
        if has_bq:
            bq = sb.tile([128, 2], F32, name="bq")
            nc.sync.dma_start(bq[:], bq_d[:, :])
        if has_bk:
            bk = sb.tile([128, 2], F32, name="bk")
            nc.sync.dma_start(bk[:], bk_d[:, :])
        # x as 2 (row-chunk ci) x 4 (1024-col blocks) tiles
        xt = [[sb.tile([128, 1024], BF16, name=f"x{ci}_{b}") for b in range(4)]
              for ci in range(2)]
        for b in range(4):
            for ci in range(2):
                nc.sync.dma_start(xt[ci][b][:],
                                  x_d[ci * 128:(ci + 1) * 128,
                                      b * 1024:(b + 1) * 1024])

        # spt prefetch on the (otherwise idle) GPSIMD DMA ring, pair granular
        spt_tiles = {}

        def load_spt(p):
            t = spt_pool.tile([128, 2, NQ], BF16, tag="spt")
            nc.gpsimd.dma_start(t[:], spt_t4[p])
            spt_tiles[p] = t

        for p in range(3):
            load_spt(p)

        # late inputs (needed only after the attention loop): tiles declared
        # here, DMAs issued mid-loop so they don't compete with x/spt early
        xqr = sb.tile([128, 2, NQ], F32, name="xqr")
        w1t = sb.tile([128, 2, 128], BF16, name="w1t")
        w2t = sb.tile([128, 128], BF16, name="w2t")
        w3t = sb.tile([128, D], BF16, name="w3t")
        if has_b12:
            b1 = sb.tile([128, 1], F32, name="b1")
            b2 = sb.tile([128, 1], F32, name="b2")
        if has_bv:
            bv = sb.tile([128, 2], F32, name="bv")
        if has_b3:
            b3 = sb.tile([128, 2], F32, name="b3")

        def load_late_inputs():
            nc.gpsimd.dma_start(xqr[:], xqr3_d[:])
            nc.gpsimd.dma_start(w1t[:], w13_d[:])
            nc.gpsimd.dma_start(w2t[:], w2t_d[:, :])
            nc.gpsimd.dma_start(w3t[:], w3t_d[:, :])
            if has_b12:
                nc.gpsimd.dma_start(b1[:], b1_d[:, :])
                nc.gpsimd.dma_start(b2[:], b2_d[:, :])
            if has_bv:
                nc.gpsimd.dma_start(bv[:], bv_d[:, :])
            if has_b3:
                nc.gpsimd.dma_start(b3[:], b3_d[:, :])

        # persistent SBUF state; K/V^T as per-512-key-block tiles so the
        # just-in-time projection writes don't serialize against loop reads
        kb = [sb.tile([128, 2, NQ], BF16, name=f"k{ib}") for ib in range(8)]
        q_sb = sb.tile([128, 2, NQ], BF16, name="q")     # (ch-in-co, co, q)
        vtb = [sb.tile([128, 4, H, VTS], BF16, name=f"vt{ib}")
               for ib in range(8)]
        for ib in range(8):
            nc.gpsimd.memset(vtb[ib][:, :, :, 64:65], 1.0)
        msg = [sb.tile([128, NQ], BF16, name=f"msg{co}") for co in range(2)]

        # ---- Q projection (first real PE work; warms HAM) ----
        qps = ps.tile([128, 2, NQ], F32, tag="t")
        for co in range(2):
            for ci in range(2):
                nc.tensor.matmul(qps[:, co, :],
                                 wqt[:, ci, co * 128:(co + 1) * 128],
                                 xq[:, ci, :],
                                 start=(ci == 0), stop=(ci == 1))
        if has_bq:
            for co in range(2):
                nc.scalar.activation(q_sb[:, co, :], qps[:, co, :], Id,
                                     bias=bq[:, co:co + 1])
        else:
            nc.scalar.copy(q_sb[:], qps[:])

        def proj_block(ib):
            # K and V^T projection for 512-key block ib (keys ib*512 ..)
            b, off = ib // 2, (ib % 2) * 512
            kps = ps.tile([128, 2, NQ], F32, tag="t")
            for co in range(2):
                for ci in range(2):
                    nc.tensor.matmul(kps[:, co, :],
                                     wkt[:, ci, co * 128:(co + 1) * 128],
                                     xt[ci][b][:, off:off + 512],
                                     start=(ci == 0), stop=(ci == 1))
            if has_bk:
                for co in range(2):
                    nc.scalar.activation(kb[ib][:, co, :], kps[:, co, :], Id,
                                         bias=bk[:, co:co + 1])
            else:
                # K evacuation on ACT; V evacuation below on DVE (balance)
                nc.scalar.copy(kb[ib][:], kps[:])
            vps = ps.tile([128, 2, NQ], F32, tag="t")
            vps4 = vps[:].rearrange("p a o -> p (a o)").rearrange(
                "p (w c) -> p w c", c=D)
            for w in range(4):
                icol = slice(off + w * 128, off + w * 128 + 128)
                for ci in range(2):
                    nc.tensor.matmul(vps4[:, w, :],
                                     xt[ci][b][:, icol],
                                     wvt[:, ci, :],
                                     start=(ci == 0), stop=(ci == 1))
            vdst = vtb[ib][:, :, :, 0:64]
            vsrc = vps4.rearrange("p w (h c) -> p w h c", h=H)
            nc.vector.tensor_copy(vdst, vsrc)

        # all projections upfront: keeps the PE dense early (HAM warm-up)
        # and the attention loop free of PSUM-pool contention
        for ib in range(8):
            proj_block(ib)

        # message-MLP accumulators (live across the whole loop)
        mps = [ps_m.tile([65, NQ], F32, name=f"mps{h}") for h in range(H)]

        def emit_half(p, u, el, spt_t):
            # scores + mask-mult for iteration it = 2p+u; each score tile's
            # DVE consumer is emitted before the pool hands its buffer out
            # again (bufs=2 invariant)
            it = 2 * p + u
            sbase = spt_t[:, u, :]
            spt_b = bass.AP(tensor=sbase.tensor, offset=sbase.offset,
                            ap=[list(sbase.ap[0]), [0, 2], list(sbase.ap[1])])
            tiles = []
            for hp in range(2):
                sps = ps.tile([128, 2, NQ], F32, tag="t")
                for j in range(2):
                    ro = j * 64
                    nc.tensor.matmul(
                        sps[:, j, :],
                        kb[it // 4][ro:ro + 64, hp,
                                    (it % 4) * 128:(it % 4) * 128 + 128],
                        q_sb[ro:ro + 64, hp, :],
                        start=True, stop=True)
                tiles.append(sps)
            for hp in range(2):
                o = 4 * u + 2 * hp
                nc.vector.tensor_mul(el[:, o:o + 2, :], tiles[hp][:], spt_b)

        def emit_msgs_it(it, e2, last=False):
            # message matmuls for one iteration (4 heads); on the last
            # iteration the per-head denominator gather chases each head's
            # final matmul
            u = it % 2
            for h in range(H):
                nc.tensor.matmul(mps[h][:], vtb[it // 4][:, it % 4, h, 0:65],
                                 e2[:, 4 * u + h, :],
                                 start=(it == 0), stop=(it == NIT - 1))
                if last:
                    # engine APs must start at a quadrant-aligned partition
                    nc.scalar.copy(dh4[32 * h:32 * h + 1, :], mps[h][64:65, :])

        # ---- main loop over key-chunk pairs; per-iteration interleave of
        # scores/mults/messages keeps every engine continuously busy ----
        dh4 = sb.tile([128, NQ], F32, name="dh4")
        nc.gpsimd.memset(dh4[:], 1.0)
        pend = None  # e2 tile of the previous pair
        for p in range(NPAIR):
            if p + 3 < NPAIR:
                load_spt(p + 3)
            if p == 4:
                load_late_inputs()
            spt_t = spt_tiles.pop(p)
            el = el_pool.tile([128, 8, NQ], BF16, tag="el")
            for u in range(2):
                emit_half(p, u, el, spt_t)
                if pend is not None:
                    emit_msgs_it(2 * (p - 1) + u, pend)
            e2 = e2_pool.tile([128, 8, NQ], BF16, tag="e2")
            nc.scalar.activation(e2[:], el[:], Exp)
            pend = e2
        emit_msgs_it(NIT - 2, pend)
        emit_msgs_it(NIT - 1, pend, last=True)

        # ---- softmax normalization: one batched reciprocal over all four
        # denominator rows (at partitions 0/32/64/96), then per-head
        # broadcast from partition 0 ----
        scr = sb.tile([128, NQ], F32, name="scr")
        rb4 = sb.tile([128, NQ], F32, name="rb4")
        nc.vector.reciprocal_approx_accurate(out=rb4[:], in_=dh4[:],
                                             scratch=scr[:])
        for h in range(H):
            co, ro = h // 2, (h % 2) * 64
            rbh = sb.tile([1, NQ], F32, name=f"rbh{h}")
            nc.scalar.copy(rbh[:], rb4[32 * h:32 * h + 1, :])
            dbc = sb.tile([64, NQ], F32, name=f"dbc{h}")
            nc.gpsimd.partition_broadcast(dbc[:], rbh[:], channels=64)
            nc.vector.tensor_mul(msg[co][ro:ro + 64, :], mps[h][0:64, :],
                                 dbc[:])
            if has_bv:
                nc.scalar.activation(msg[co][ro:ro + 64, :],
                                     msg[co][ro:ro + 64, :], Id,
                                     bias=bv[ro:ro + 64, co:co + 1])

        # ---- message MLP + residual ----
        u1 = ps.tile([128, 2, NQ], F32, tag="t")
        for ci in range(2):
            nc.tensor.matmul(u1[:, 0, :], w1t[:, ci, :], msg[ci][:],
                             start=(ci == 0), stop=(ci == 1))
        h1 = sb.tile([128, NQ], BF16, name="h1")
        if has_b12:
            nc.scalar.activation(h1[:], u1[:, 0, :], Relu, bias=b1[:, 0:1])
        else:
            nc.scalar.activation(h1[:], u1[:, 0, :], Relu)
        u2 = ps.tile([128, 2, NQ], F32, tag="t")
        nc.tensor.matmul(u2[:, 0, :], w2t[:], h1[:], start=True, stop=True)
        h2 = sb.tile([128, NQ], BF16, name="h2")
        if has_b12:
            nc.scalar.activation(h2[:], u2[:, 0, :], Relu, bias=b2[:, 0:1])
        else:
            nc.scalar.activation(h2[:], u2[:, 0, :], Relu)
        for co in range(2):
            u3 = ps.tile([128, 2, NQ], F32, tag="t")
            nc.tensor.matmul(u3[:, 0, :], w3t[:, co * 128:(co + 1) * 128],
                             h2[:], start=True, stop=True)
            ot = sb.tile([128, NQ], F32, name=f"ot{co}")
            if has_b3:
                tb = sb.tile([128, NQ], F32, name=f"tb{co}")
                nc.scalar.activation(tb[:], u3[:, 0, :], Id,
                                     bias=b3[:, co:co + 1])
                nc.vector.tensor_add(ot[:], tb[:], xqr[:, co, :])
            else:
                nc.vector.tensor_add(ot[:], u3[:, 0, :], xqr[:, co, :])
            nc.sync.dma_start(out_d[co * 128:(co + 1) * 128, :], ot[:])

    nc.compile()
    return nc


def _prep_inputs(inputs):
    import ml_dtypes
    bf = lambda a: np.ascontiguousarray(
        np.asarray(a, dtype=np.float32).astype(ml_dtypes.bfloat16))
    f = lambda a: np.ascontiguousarray(np.asarray(a, dtype=np.float32))
    x32 = f(inputs["corr_feat_belief"][0])                  # [D, N]
    spT = np.asarray(inputs["spatial_compatibility"][0]).T  # [N(keys), N(queries)]
    Wq, bq = f(inputs["Wq"]), f(inputs["bq"])
    Wk, bk = f(inputs["Wk"]), f(inputs["bk"])
    Wv, bv = f(inputs["Wv"]), f(inputs["bv"])
    W1, b1, g1, be1 = f(inputs["W1"]), f(inputs["b1"]), f(inputs["g1"]), f(inputs["be1"])
    W2, b2, g2, be2 = f(inputs["W2"]), f(inputs["b2"]), f(inputs["g2"]), f(inputs["be2"])
    W3, b3 = f(inputs["W3"]), f(inputs["b3"])

    scale = np.float32(1.0 / np.sqrt(DH))
    s1 = (g1 / np.sqrt(np.float32(1.0) + np.float32(1e-5))).astype(np.float32)
    s2 = (g2 / np.sqrt(np.float32(1.0) + np.float32(1e-5))).astype(np.float32)

    spT_bf = bf(spT)
    x_bf = bf(x32)
    b1f = (s1 * b1 + be1).astype(np.float32)
    b2f = (s2 * b2 + be2).astype(np.float32)
    common = dict(
        x=x_bf,
        wqt=bf(Wq.T * scale),
        wkt=bf(Wk.T),
        wvt=bf(Wv.T),
        w1t=bf((W1 * s1[:, None]).T),
        w2t=bf((W2 * s2[:, None]).T),
        w3t=bf(W3.T),
        bq2=f((bq * scale).reshape(2, 128).T),
        bk2=f(bk.reshape(2, 128).T),
        bv2=f(bv.reshape(2, 128).T),
        b1f=f(b1f.reshape(128, 1)),
        b2f=f(b2f.reshape(128, 1)),
        b32=f(b3.reshape(2, 128).T),
    )
    in_maps = []
    for m in range(NCORES):
        sl = slice(m * NQ, (m + 1) * NQ)
        im = dict(common)
        im["xq"] = np.ascontiguousarray(x_bf[:, sl])
        im["xqr"] = f(x32[:, sl])
        im["spt"] = np.ascontiguousarray(spT_bf[:, sl])
        in_maps.append(im)
    flags = (bool(np.any(bq != 0)), bool(np.any(bk != 0)),
             bool(np.any(bv != 0)), bool(np.any(b3 != 0)),
             bool(np.any(b1f != 0) or np.any(b2f != 0)))
    return in_maps, flags


def _run(inputs, trace=False):
    from concourse.bass_utils import run_bass_kernel_spmd
    in_maps, flags = _prep_inputs(inputs)
    if flags not in _CACHE:
        _CACHE[flags] = _build(*flags)
    nc = _CACHE[flags]
    res = run_bass_kernel_spmd(nc, in_maps, core_ids=list(range(NCORES)),
                               trace=trace)
    out = np.concatenate([res.results[m]["out"] for m in range(NCORES)],
                         axis=1)[None]
    return np.ascontiguousarray(out.astype(np.float32)), res


def kernel(**inputs):
    out, _ = _run(inputs, trace=False)
    return out


# revision 17
# speedup vs baseline: 1.1529x; 1.1529x over previous
"""Bass/Tile TRN2 kernel for a non-local attention block (BaseNonLocalBlock).

Contract: kernel(**inputs) takes the FULL inputs of the nn.Module problem
(B=1, D=256, H=4, N=4096) and returns the FULL output [1, 256, 4096].

Sharding: query columns of the N x N attention are split across the 8
NeuronCores (512 queries per core). K/V projections are computed
redundantly on every core (cheap); each core produces its own output
column slice and the host concatenates.

Per-core algorithm (flash-attention style, scores never hit HBM):
  Q = (Wq/8) @ xq                      [256, 512]  (1/sqrt(DH) folded in)
  per 512-key block ib (projected JUST-IN-TIME, interleaved into the
  attention loop two pairs ahead):
    K[:, ib]  = Wk @ x[:, ib]          -> k_sb[128, 2, 4096] (co-major)
    V_T[ib]   = x[:, ib]^T @ Wv^T      -> vt[128, it, h, 68] (ones col 64)
  attention loop over PAIRS of 128-key chunks (it = 2p, 2p+1):
    S_T = K_h^T @ Q_h                  [128, 2, 512] PSUM per (it, hp)
    el  = spt * S_T                    DVE mult (4 per pair)
    e2  = exp(el)                      ONE ACT exp, FD=4096 (batched pair)
    msgs for pair p-1: mps[h] += V_T^T @ e2   (PSUM accum; row 64 = denom)
  tail: per-head denom gather -> one DVE reciprocal -> gpsimd broadcast
  -> normalize; out = xq + W3 @ relu(bn2(W2 @ relu(bn1(W1 @ msg))))

Matmul operands are bf16; accumulation fp32 in PSUM; residual reads a
separate fp32 copy of x so the dominant term is exact.
"""

import numpy as np
from contextlib import ExitStack

D = 256
N = 4096
NQ = 512          # queries per core
H = 4
DH = 64
NCORES = 8
NIT = N // 128    # 32 key chunks
NPAIR = NIT // 2  # 16 chunk pairs
VTS = 68          # padded per-head stride in the V_T-aug tile

_CACHE = {}


def _build(has_bq, has_bk, has_bv, has_b3, has_b12):
    import concourse.bass as bass
    import concourse.tile as tile
    from concourse import bacc, mybir

    F32 = mybir.dt.float32
    BF16 = mybir.dt.bfloat16
    Id = mybir.ActivationFunctionType.Identity
    Exp = mybir.ActivationFunctionType.Exp
    Relu = mybir.ActivationFunctionType.Relu

    nc = bacc.Bacc("TRN2", target_bir_lowering=False, debug=False,
                   num_devices=NCORES)

    # DRAM I/O (per core)
    x_d = nc.dram_tensor("x", [D, N], BF16, kind="ExternalInput").ap()
    xq_d = nc.dram_tensor("xq", [D, NQ], BF16, kind="ExternalInput").ap()
    xqr_d = nc.dram_tensor("xqr", [D, NQ], F32, kind="ExternalInput").ap()
    spt_d = nc.dram_tensor("spt", [N, NQ], BF16, kind="ExternalInput").ap()
    wqt_d = nc.dram_tensor("wqt", [D, D], BF16, kind="ExternalInput").ap()
    wkt_d = nc.dram_tensor("wkt", [D, D], BF16, kind="ExternalInput").ap()
    wvt_d = nc.dram_tensor("wvt", [D, D], BF16, kind="ExternalInput").ap()
    w1t_d = nc.dram_tensor("w1t", [D, 128], BF16, kind="ExternalInput").ap()
    w2t_d = nc.dram_tensor("w2t", [128, 128], BF16, kind="ExternalInput").ap()
    w3t_d = nc.dram_tensor("w3t", [128, D], BF16, kind="ExternalInput").ap()
    bq_d = nc.dram_tensor("bq2", [128, 2], F32, kind="ExternalInput").ap()
    bk_d = nc.dram_tensor("bk2", [128, 2], F32, kind="ExternalInput").ap()
    bv_d = nc.dram_tensor("bv2", [128, 2], F32, kind="ExternalInput").ap()
    b1_d = nc.dram_tensor("b1f", [128, 1], F32, kind="ExternalInput").ap()
    b2_d = nc.dram_tensor("b2f", [128, 1], F32, kind="ExternalInput").ap()
    b3_d = nc.dram_tensor("b32", [128, 2], F32, kind="ExternalInput").ap()
    out_d = nc.dram_tensor("out", [D, NQ], F32, kind="ExternalOutput").ap()

    # pair-granular view of spt: pair p, partition, (u, queries)
    spt_t4 = spt_d.rearrange("(t u p) o -> t p u o", u=2, p=128)
    # (ci, p) views of the [256, .] weight matrices
    wq3_d = wqt_d.rearrange("(c p) o -> p c o", p=128)
    wk3_d = wkt_d.rearrange("(c p) o -> p c o", p=128)
    wv3_d = wvt_d.rearrange("(c p) o -> p c o", p=128)
    w13_d = w1t_d.rearrange("(c p) o -> p c o", p=128)
    xq3_d = xq_d.rearrange("(c p) o -> p c o", p=128)
    xqr3_d = xqr_d.rearrange("(c p) o -> p c o", p=128)

    with tile.TileContext(nc) as tc, ExitStack() as ctx:
        sb = ctx.enter_context(tc.tile_pool(name="sb", bufs=1))
        spt_pool = ctx.enter_context(tc.tile_pool(name="sptp", bufs=4))
        el_pool = ctx.enter_context(tc.tile_pool(name="elp", bufs=3))
        e2_pool = ctx.enter_context(tc.tile_pool(name="e2p", bufs=3))
        pj_ctx = ExitStack()
        pj = pj_ctx.enter_context(tc.tile_pool(name="pj", bufs=3, space="PSUM"))

        # ---- ACT exp-table preload (overlaps the DMA ramp) ----
        warm = sb.tile([1, 2], F32, name="warm")
        nc.vector.memset(warm[:], 0.0)
        nc.scalar.activation(warm[:], warm[:], Exp)

        # ---- PE warmup: tiny matmuls with no DMA deps so HAM unthrottles
        # during the DMA ramp (dummy operands; result never read) ----
        wsmall = sb.tile([128, 64], BF16, name="wsmall")
        nc.vector.memset(wsmall[:].bitcast(F32)[:, 0:32], 0.0)
        wps = pj.tile([128, 2, NQ], F32, tag="t")
        for r in range(32):
            nc.tensor.matmul(wps[0:64, 0, 0:64], wsmall[:], wsmall[:],
                             start=True, stop=True)

        # ---- weights + inputs: big DMAs, earliest-needed first ----
        wqt = sb.tile([128, 2, D], BF16, name="wqt")
        wkt = sb.tile([128, 2, D], BF16, name="wkt")
        wvt = sb.tile([128, 2, D], BF16, name="wvt")
        xq = sb.tile([128, 2, NQ], BF16, name="xq")
        nc.sync.dma_start(wqt[:], wq3_d[:])
        nc.sync.dma_start(xq[:], xq3_d[:])
        nc.sync.dma_start(wkt[:], wk3_d[:])
        nc.sync.dma_start(wvt[:], wv3_d[:])
        if has_bq:
            bq = sb.tile([128, 2], F32, name="bq")
            nc.sync.dma_start(bq[:], bq_d[:, :])
        if has_bk:
            bk = sb.tile([128, 2], F32, name="bk")
            nc.sync.dma_start(bk[:], bk_d[:, :])
        # x as 2 (row-chunk ci) x 4 (1024-col blocks) tiles
        xt = [[sb.tile([128, 1024], BF16, name=f"x{ci}_{b}") for b in range(4)]
              for ci in range(2)]
        for b in range(4):
            for ci in range(2):
                nc.sync.dma_start(xt[ci][b][:],
                                  x_d[ci * 128:(ci + 1) * 128,
                                      b * 1024:(b + 1) * 1024])

        # spt prefetch on the (otherwise idle) GPSIMD DMA ring, pair granular
        spt_tiles = {}

        def load_spt(p):
            t = spt_pool.tile([128, 2, NQ], BF16, tag="spt")
            nc.gpsimd.dma_start(t[:], spt_t4[p])
            spt_tiles[p] = t

        for p in range(3):
            load_spt(p)

        # late inputs (needed only after the attention loop): tiles declared
        # here, DMAs issued mid-loop so they don't compete with x/spt early
        xqr = sb.tile([128, 2, NQ], F32, name="xqr")
        w1t = sb.tile([128, 2, 128], BF16, name="w1t")
        w2t = sb.tile([128, 128], BF16, name="w2t")
        w3t = sb.tile([128, D], BF16, name="w3t")
        if has_b12:
            b1 = sb.tile([128, 1], F32, name="b1")
            b2 = sb.tile([128, 1], F32, name="b2")
        if has_bv:
            bv = sb.tile([128, 2], F32, name="bv")
        if has_b3:
            b3 = sb.tile([128, 2], F32, name="b3")

        def load_late_inputs():
            nc.gpsimd.dma_start(xqr[:], xqr3_d[:])
            nc.gpsimd.dma_start(w1t[:], w13_d[:])
            nc.gpsimd.dma_start(w2t[:], w2t_d[:, :])
            nc.gpsimd.dma_start(w3t[:], w3t_d[:, :])
            if has_b12:
                nc.gpsimd.dma_start(b1[:], b1_d[:, :])
                nc.gpsimd.dma_start(b2[:], b2_d[:, :])
            if has_bv:
                nc.gpsimd.dma_start(bv[:], bv_d[:, :])
            if has_b3:
                nc.gpsimd.dma_start(b3[:], b3_d[:, :])

        # persistent SBUF state; K/V^T as per-512-key-block tiles so the
        # just-in-time projection writes don't serialize against loop reads
        kb = [sb.tile([128, 2, NQ], BF16, name=f"k{ib}") for ib in range(8)]
        q_sb = sb.tile([128, 2, NQ], BF16, name="q")     # (ch-in-co, co, q)
        vtb = [sb.tile([128, 4, H, VTS], BF16, name=f"vt{ib}")
               for ib in range(8)]
        for ib in range(8):
            nc.gpsimd.memset(vtb[ib][:, :, :, 64:65], 1.0)
        msg = [sb.tile([128, NQ], BF16, name=f"msg{co}") for co in range(2)]

        # ---- Q projection (first real PE work; warms HAM) ----
        qps = pj.tile([128, 2, NQ], F32, tag="t")
        for co in range(2):
            for ci in range(2):
                nc.tensor.matmul(qps[:, co, :],
                                 wqt[:, ci, co * 128:(co + 1) * 128],
                                 xq[:, ci, :],
                                 start=(ci == 0), stop=(ci == 1))
        if has_bq:
            for co in range(2):
                nc.scalar.activation(q_sb[:, co, :], qps[:, co, :], Id,
                                     bias=bq[:, co:co + 1])
        else:
            nc.scalar.copy(q_sb[:], qps[:])

        def proj_block(ib):
            # K and V^T projection for 512-key block ib (keys ib*512 ..)
            b, off = ib // 2, (ib % 2) * 512
            kps = pj.tile([128, 2, NQ], F32, tag="t")
            for co in range(2):
                for ci in range(2):
                    nc.tensor.matmul(kps[:, co, :],
                                     wkt[:, ci, co * 128:(co + 1) * 128],
                                     xt[ci][b][:, off:off + 512],
                                     start=(ci == 0), stop=(ci == 1))
            if has_bk:
                for co in range(2):
                    nc.scalar.activation(kb[ib][:, co, :], kps[:, co, :], Id,
                                         bias=bk[:, co:co + 1])
            else:
                # K evacuation on ACT; V evacuation below on DVE (balance)
                nc.scalar.copy(kb[ib][:], kps[:])
            vps = pj.tile([128, 2, NQ], F32, tag="t")
            vps4 = vps[:].rearrange("p a o -> p (a o)").rearrange(
                "p (w c) -> p w c", c=D)
            for w in range(4):
                icol = slice(off + w * 128, off + w * 128 + 128)
                for ci in range(2):
                    nc.tensor.matmul(vps4[:, w, :],
                                     xt[ci][b][:, icol],
                                     wvt[:, ci, :],
                                     start=(ci == 0), stop=(ci == 1))
            vdst = vtb[ib][:, :, :, 0:64]
            vsrc = vps4.rearrange("p w (h c) -> p w h c", h=H)
            nc.vector.tensor_copy(vdst, vsrc)

        # all projections upfront: keeps the PE dense early (HAM warm-up)
        # and the attention loop free of PSUM-pool contention
        for ib in range(8):
            proj_block(ib)

        # projection PSUM pool closes; loop pools take its banks
        pj_ctx.close()
        ps = ctx.enter_context(tc.tile_pool(name="ps", bufs=2, space="PSUM"))
        ps_m = ctx.enter_context(tc.tile_pool(name="psm", bufs=1, space="PSUM"))

        # message-MLP accumulators (live across the whole loop)
        mps = [ps_m.tile([65, NQ], F32, name=f"mps{h}") for h in range(H)]

        def emit_scores_mults(it, el, spt_t):
            # scores + mask-mult for iteration it; each score tile's DVE
            # consumer is emitted before the pool hands its buffer out again
            sbase = spt_t[:, it % 2, :]
            spt_b = bass.AP(tensor=sbase.tensor, offset=sbase.offset,
                            ap=[list(sbase.ap[0]), [0, 2], list(sbase.ap[1])])
            tiles = []
            for hp in range(2):
                sps = ps.tile([128, 2, NQ], F32, tag="t")
                for j in range(2):
                    ro = j * 64
                    nc.tensor.matmul(
                        sps[:, j, :],
                        kb[it // 4][ro:ro + 64, hp,
                                    (it % 4) * 128:(it % 4) * 128 + 128],
                        q_sb[ro:ro + 64, hp, :],
                        start=True, stop=True)
                tiles.append(sps)
            for hp in range(2):
                nc.vector.tensor_mul(el[:, 2 * hp:2 * hp + 2, :],
                                     tiles[hp][:], spt_b)

        def emit_msgs_it(it, e2, last=False):
            # message matmuls for one iteration (4 heads); on the last
            # iteration the per-head denominator gather chases each head's
            # final matmul
            for h in range(H):
                nc.tensor.matmul(mps[h][:], vtb[it // 4][:, it % 4, h, 0:65],
                                 e2[:, h, :],
                                 start=(it == 0), stop=(it == NIT - 1))
                if last:
                    # engine APs must start at a quadrant-aligned partition
                    nc.scalar.copy(dh4[32 * h:32 * h + 1, :], mps[h][64:65, :])

        # ---- main loop: per-iteration scores -> mult -> exp, messages one
        # iteration behind, so every engine stays continuously busy ----
        dh4 = sb.tile([128, NQ], F32, name="dh4")
        nc.gpsimd.memset(dh4[:], 1.0)
        pend = None  # (it, e2 tile) of the previous iteration
        for it in range(NIT):
            if it % 2 == 0:
                p = it // 2
                if p + 3 < NPAIR:
                    load_spt(p + 3)
                if p == 4:
                    load_late_inputs()
                spt_t = spt_tiles[p]
            el = el_pool.tile([128, 4, NQ], BF16, tag="el")
            emit_scores_mults(it, el, spt_t)
            if pend is not None:
                emit_msgs_it(pend[0], pend[1])
            if it % 2 == 1:
                spt_tiles.pop(it // 2)
            e2 = e2_pool.tile([128, 4, NQ], BF16, tag="e2")
            nc.scalar.activation(e2[:], el[:], Exp)
            pend = (it, e2)
        emit_msgs_it(NIT - 1, pend[1], last=True)

        # ---- softmax normalization: one batched reciprocal over all four
        # denominator rows (at partitions 0/32/64/96), then per-head
        # broadcast from partition 0 ----
        scr = sb.tile([128, NQ], F32, name="scr")
        rb4 = sb.tile([128, NQ], F32, name="rb4")
        nc.vector.reciprocal_approx_accurate(out=rb4[:], in_=dh4[:],
                                             scratch=scr[:])
        for h in range(H):
            co, ro = h // 2, (h % 2) * 64
            rbh = sb.tile([1, NQ], F32, name=f"rbh{h}")
            nc.scalar.copy(rbh[:], rb4[32 * h:32 * h + 1, :])
            dbc = sb.tile([64, NQ], F32, name=f"dbc{h}")
            nc.gpsimd.partition_broadcast(dbc[:], rbh[:], channels=64)
            nc.vector.tensor_mul(msg[co][ro:ro + 64, :], mps[h][0:64, :],
                                 dbc[:])
            if has_bv:
                nc.scalar.activation(msg[co][ro:ro + 64, :],
                                     msg[co][ro:ro + 64, :], Id,
                                     bias=bv[ro:ro + 64, co:co + 1])

        # ---- message MLP + residual ----
        u1 = ps.tile([128, 2, NQ], F32, tag="t")
        for ci in range(2):
            nc.tensor.matmul(u1[:, 0, :], w1t[:, ci, :], msg[ci][:],
                             start=(ci == 0), stop=(ci == 1))
        h1 = sb.tile([128, NQ], BF16, name="h1")
        if has_b12:
            nc.scalar.activation(h1[:], u1[:, 0, :], Relu, bias=b1[:, 0:1])
        else:
            nc.scalar.activation(h1[:], u1[:, 0, :], Relu)
        u2 = ps.tile([128, 2, NQ], F32, tag="t")
        nc.tensor.matmul(u2[:, 0, :], w2t[:], h1[:], start=True, stop=True)
        h2 = sb.tile([128, NQ], BF16, name="h2")
        if has_b12:
            nc.scalar.activation(h2[:], u2[:, 0, :], Relu, bias=b2[:, 0:1])
        else:
            nc.scalar.activation(h2[:], u2[:, 0, :], Relu)
        for co in range(2):
            u3 = ps.tile([128, 2, NQ], F32, tag="t")
            nc.tensor.matmul(u3[:, 0, :], w3t[:, co * 128:(co + 1) * 128],
                             h2[:], start=True, stop=True)
            ot = sb.tile([128, NQ], F32, name=f"ot{co}")
            if has_b3:
                tb = sb.tile([128, NQ], F32, name=f"tb{co}")
                nc.scalar.activation(tb[:], u3[:, 0, :], Id,
                                     bias=b3[:, co:co + 1])
                nc.vector.tensor_add(ot[:], tb[:], xqr[:, co, :])
            else:
                nc.vector.tensor_add(ot[:], u3[:, 0, :], xqr[:, co, :])
            nc.sync.dma_start(out_d[co * 128:(co + 1) * 128, :], ot[:])

    nc.compile()
    return nc


def _prep_inputs(inputs):
    import ml_dtypes
    bf = lambda a: np.ascontiguousarray(
        np.asarray(a, dtype=np.float32).astype(ml_dtypes.bfloat16))
    f = lambda a: np.ascontiguousarray(np.asarray(a, dtype=np.float32))
    x32 = f(inputs["corr_feat_belief"][0])                  # [D, N]
    spT = np.asarray(inputs["spatial_compatibility"][0]).T  # [N(keys), N(queries)]
    Wq, bq = f(inputs["Wq"]), f(inputs["bq"])
    Wk, bk = f(inputs["Wk"]), f(inputs["bk"])
    Wv, bv = f(inputs["Wv"]), f(inputs["bv"])
    W1, b1, g1, be1 = f(inputs["W1"]), f(inputs["b1"]), f(inputs["g1"]), f(inputs["be1"])
    W2, b2, g2, be2 = f(inputs["W2"]), f(inputs["b2"]), f(inputs["g2"]), f(inputs["be2"])
    W3, b3 = f(inputs["W3"]), f(inputs["b3"])

    scale = np.float32(1.0 / np.sqrt(DH))
    s1 = (g1 / np.sqrt(np.float32(1.0) + np.float32(1e-5))).astype(np.float32)
    s2 = (g2 / np.sqrt(np.float32(1.0) + np.float32(1e-5))).astype(np.float32)

    spT_bf = bf(spT)
    x_bf = bf(x32)
    b1f = (s1 * b1 + be1).astype(np.float32)
    b2f = (s2 * b2 + be2).astype(np.float32)
    common = dict(
        x=x_bf,
        wqt=bf(Wq.T * scale),
        wkt=bf(Wk.T),
        wvt=bf(Wv.T),
        w1t=bf((W1 * s1[:, None]).T),
        w2t=bf((W2 * s2[:, None]).T),
        w3t=bf(W3.T),
        bq2=f((bq * scale).reshape(2, 128).T),
        bk2=f(bk.reshape(2, 128).T),
        bv2=f(bv.reshape(2, 128).T),
        b1f=f(b1f.reshape(128, 1)),
        b2f=f(b2f.reshape(128, 1)),
        b32=f(b3.reshape(2, 128).T),
    )
    in_maps = []
    for m in range(NCORES):
        sl = slice(m * NQ, (m + 1) * NQ)
        im = dict(common)
        im["xq"] = np.ascontiguousarray(x_bf[:, sl])
        im["xqr"] = f(x32[:, sl])
        im["spt"] = np.ascontiguousarray(spT_bf[:, sl])
        in_maps.append(im)
    flags = (bool(np.any(bq != 0)), bool(np.any(bk != 0)),
             bool(np.any(bv != 0)), bool(np.any(b3 != 0)),
             bool(np.any(b1f != 0) or np.any(b2f != 0)))
    return in_maps, flags


def _run(inputs, trace=False):
    from concourse.bass_utils import run_bass_kernel_spmd
    in_maps, flags = _prep_inputs(inputs)
    if flags not in _CACHE:
        _CACHE[flags] = _build(*flags)
    nc = _CACHE[flags]
    res = run_bass_kernel_spmd(nc, in_maps, core_ids=list(range(NCORES)),
                               trace=trace)
    out = np.concatenate([res.results[m]["out"] for m in range(NCORES)],
                         axis=1)[None]
    return np.ascontiguousarray(out.astype(np.float32)), res


def kernel(**inputs):
    out, _ = _run(inputs, trace=False)
    return out


# revision 19
# speedup vs baseline: 1.3565x; 1.1766x over previous
"""Bass/Tile TRN2 kernel for a non-local attention block (BaseNonLocalBlock).

Contract: kernel(**inputs) takes the FULL inputs of the nn.Module problem
(B=1, D=256, H=4, N=4096) and returns the FULL output [1, 256, 4096].

Sharding: query columns of the N x N attention are split across the 8
NeuronCores (512 queries per core). K/V projections are computed
redundantly on every core (cheap); each core produces its own output
column slice and the host concatenates.

Per-core algorithm (flash-attention style, scores never hit HBM):
  Q = (Wq/8) @ xq                      [256, 512]  (1/sqrt(DH) folded in)
  per 512-key block ib (projected JUST-IN-TIME, interleaved into the
  attention loop two pairs ahead):
    K[:, ib]  = Wk @ x[:, ib]          -> k_sb[128, 2, 4096] (co-major)
    V_T[ib]   = x[:, ib]^T @ Wv^T      -> vt[128, it, h, 68] (ones col 64)
  attention loop over PAIRS of 128-key chunks (it = 2p, 2p+1):
    S_T = K_h^T @ Q_h                  [128, 2, 512] PSUM per (it, hp)
    el  = spt * S_T                    DVE mult (4 per pair)
    e2  = exp(el)                      ONE ACT exp, FD=4096 (batched pair)
    msgs for pair p-1: mps[h] += V_T^T @ e2   (PSUM accum; row 64 = denom)
  tail: per-head denom gather -> one DVE reciprocal -> gpsimd broadcast
  -> normalize; out = xq + W3 @ relu(bn2(W2 @ relu(bn1(W1 @ msg))))

Matmul operands are bf16; accumulation fp32 in PSUM; residual reads a
separate fp32 copy of x so the dominant term is exact.
"""

import numpy as np
from contextlib import ExitStack

D = 256
N = 4096
NQ = 512          # queries per core
H = 4
DH = 64
NCORES = 8
NIT = N // 128    # 32 key chunks
NPAIR = NIT // 2  # 16 chunk pairs
VTS = 68          # padded per-head stride in the V_T-aug tile

_CACHE = {}


def _build(has_bq, has_bk, has_bv, has_b3, has_b12):
    import concourse.bass as bass
    import concourse.tile as tile
    from concourse import bacc, mybir

    F32 = mybir.dt.float32
    BF16 = mybir.dt.bfloat16
    Id = mybir.ActivationFunctionType.Identity
    Exp = mybir.ActivationFunctionType.Exp
    Relu = mybir.ActivationFunctionType.Relu

    nc = bacc.Bacc("TRN2", target_bir_lowering=False, debug=False,
                   num_devices=NCORES)

    # DRAM I/O (per core)
    x_d = nc.dram_tensor("x", [D, N], BF16, kind="ExternalInput").ap()
    xq_d = nc.dram_tensor("xq", [D, NQ], BF16, kind="ExternalInput").ap()
    xqr_d = nc.dram_tensor("xqr", [D, NQ], F32, kind="ExternalInput").ap()
    spt_d = nc.dram_tensor("spt", [N, NQ], BF16, kind="ExternalInput").ap()
    wqt_d = nc.dram_tensor("wqt", [D, D], BF16, kind="ExternalInput").ap()
    wkt_d = nc.dram_tensor("wkt", [D, D], BF16, kind="ExternalInput").ap()
    wvt_d = nc.dram_tensor("wvt", [D, D], BF16, kind="ExternalInput").ap()
    w1t_d = nc.dram_tensor("w1t", [D, 128], BF16, kind="ExternalInput").ap()
    w2t_d = nc.dram_tensor("w2t", [128, 128], BF16, kind="ExternalInput").ap()
    w3t_d = nc.dram_tensor("w3t", [128, D], BF16, kind="ExternalInput").ap()
    bq_d = nc.dram_tensor("bq2", [128, 2], F32, kind="ExternalInput").ap()
    bk_d = nc.dram_tensor("bk2", [128, 2], F32, kind="ExternalInput").ap()
    bv_d = nc.dram_tensor("bv2", [128, 2], F32, kind="ExternalInput").ap()
    b1_d = nc.dram_tensor("b1f", [128, 1], F32, kind="ExternalInput").ap()
    b2_d = nc.dram_tensor("b2f", [128, 1], F32, kind="ExternalInput").ap()
    b3_d = nc.dram_tensor("b32", [128, 2], F32, kind="ExternalInput").ap()
    out_d = nc.dram_tensor("out", [D, NQ], F32, kind="ExternalOutput").ap()

    # pair-granular view of spt: pair p, partition, (u, queries)
    spt_t4 = spt_d.rearrange("(t u p) o -> t p u o", u=2, p=128)
    # (ci, p) views of the [256, .] weight matrices
    wq3_d = wqt_d.rearrange("(c p) o -> p c o", p=128)
    wk3_d = wkt_d.rearrange("(c p) o -> p c o", p=128)
    wv3_d = wvt_d.rearrange("(c p) o -> p c o", p=128)
    w13_d = w1t_d.rearrange("(c p) o -> p c o", p=128)
    xq3_d = xq_d.rearrange("(c p) o -> p c o", p=128)
    xqr3_d = xqr_d.rearrange("(c p) o -> p c o", p=128)

    with tile.TileContext(nc) as tc, ExitStack() as ctx:
        sb = ctx.enter_context(tc.tile_pool(name="sb", bufs=1))
        spt_pool = ctx.enter_context(tc.tile_pool(name="sptp", bufs=4))
        el_pool = ctx.enter_context(tc.tile_pool(name="elp", bufs=3))
        e2_pool = ctx.enter_context(tc.tile_pool(name="e2p", bufs=3))
        pj_ctx = ExitStack()
        pj = pj_ctx.enter_context(tc.tile_pool(name="pj", bufs=3, space="PSUM"))

        # ---- ACT exp-table preload (overlaps the DMA ramp) ----
        warm = sb.tile([1, 2], F32, name="warm")
        nc.vector.memset(warm[:], 0.0)
        nc.scalar.activation(warm[:], warm[:], Exp)

        # ---- PE warmup: tiny matmuls with no DMA deps so HAM unthrottles
        # during the DMA ramp (dummy operands; result never read) ----
        wsmall = sb.tile([128, 64], BF16, name="wsmall")
        nc.vector.memset(wsmall[:].bitcast(F32)[:, 0:32], 0.0)
        wps = pj.tile([128, 2, NQ], F32, tag="t")
        for r in range(64):
            nc.tensor.matmul(wps[0:64, 0, 0:64], wsmall[:], wsmall[:],
                             start=True, stop=True)

        # ---- weights + inputs: big DMAs, earliest-needed first ----
        wqt = sb.tile([128, 2, D], BF16, name="wqt")
        wkt = sb.tile([128, 2, D], BF16, name="wkt")
        wvt = sb.tile([128, 2, D], BF16, name="wvt")
        xq = sb.tile([128, 2, NQ], BF16, name="xq")
        nc.sync.dma_start(wqt[:], wq3_d[:])
        nc.sync.dma_start(xq[:], xq3_d[:])
        nc.sync.dma_start(wkt[:], wk3_d[:])
        nc.sync.dma_start(wvt[:], wv3_d[:])
        if has_bq:
            bq = sb.tile([128, 2], F32, name="bq")
            nc.sync.dma_start(bq[:], bq_d[:, :])
        if has_bk:
            bk = sb.tile([128, 2], F32, name="bk")
            nc.sync.dma_start(bk[:], bk_d[:, :])
        # x as 2 (row-chunk ci) x 4 (1024-col blocks) tiles
        xt = [[sb.tile([128, 1024], BF16, name=f"x{ci}_{b}") for b in range(4)]
              for ci in range(2)]
        for b in range(4):
            for ci in range(2):
                nc.sync.dma_start(xt[ci][b][:],
                                  x_d[ci * 128:(ci + 1) * 128,
                                      b * 1024:(b + 1) * 1024])

        # spt prefetch on the (otherwise idle) GPSIMD DMA ring, pair granular
        spt_tiles = {}

        def load_spt(p):
            t = spt_pool.tile([128, 2, NQ], BF16, tag="spt")
            nc.gpsimd.dma_start(t[:], spt_t4[p])
            spt_tiles[p] = t

        for p in range(3):
            load_spt(p)

        # late inputs (needed only after the attention loop): tiles declared
        # here, DMAs issued mid-loop so they don't compete with x/spt early
        xqr = sb.tile([128, 2, NQ], F32, name="xqr")
        w1t = sb.tile([128, 2, 128], BF16, name="w1t")
        w2t = sb.tile([128, 128], BF16, name="w2t")
        w3t = sb.tile([128, D], BF16, name="w3t")
        if has_b12:
            b1 = sb.tile([128, 1], F32, name="b1")
            b2 = sb.tile([128, 1], F32, name="b2")
        if has_bv:
            bv = sb.tile([128, 2], F32, name="bv")
        if has_b3:
            b3 = sb.tile([128, 2], F32, name="b3")

        def load_late_inputs():
            nc.gpsimd.dma_start(xqr[:], xqr3_d[:])
            nc.gpsimd.dma_start(w1t[:], w13_d[:])
            nc.gpsimd.dma_start(w2t[:], w2t_d[:, :])
            nc.gpsimd.dma_start(w3t[:], w3t_d[:, :])
            if has_b12:
                nc.gpsimd.dma_start(b1[:], b1_d[:, :])
                nc.gpsimd.dma_start(b2[:], b2_d[:, :])
            if has_bv:
                nc.gpsimd.dma_start(bv[:], bv_d[:, :])
            if has_b3:
                nc.gpsimd.dma_start(b3[:], b3_d[:, :])

        # persistent SBUF state; K/V^T as per-512-key-block tiles so the
        # just-in-time projection writes don't serialize against loop reads
        kb = [sb.tile([128, 2, NQ], BF16, name=f"k{ib}") for ib in range(8)]
        q_sb = sb.tile([128, 2, NQ], BF16, name="q")     # (ch-in-co, co, q)
        vtb = [sb.tile([128, 4, H, VTS], BF16, name=f"vt{ib}")
               for ib in range(8)]
        for ib in range(8):
            nc.gpsimd.memset(vtb[ib][:, :, :, 64:65], 1.0)
        msg = [sb.tile([128, NQ], BF16, name=f"msg{co}") for co in range(2)]

        # ---- Q projection (first real PE work; warms HAM) ----
        qps = pj.tile([128, 2, NQ], F32, tag="t")
        for co in range(2):
            for ci in range(2):
                nc.tensor.matmul(qps[:, co, :],
                                 wqt[:, ci, co * 128:(co + 1) * 128],
                                 xq[:, ci, :],
                                 start=(ci == 0), stop=(ci == 1))
        if has_bq:
            for co in range(2):
                nc.scalar.activation(q_sb[:, co, :], qps[:, co, :], Id,
                                     bias=bq[:, co:co + 1])
        else:
            nc.scalar.copy(q_sb[:], qps[:])

        def proj_block(ib):
            # K and V^T projection for 512-key block ib (keys ib*512 ..)
            b, off = ib // 2, (ib % 2) * 512
            kps = pj.tile([128, 2, NQ], F32, tag="t")
            for co in range(2):
                for ci in range(2):
                    nc.tensor.matmul(kps[:, co, :],
                                     wkt[:, ci, co * 128:(co + 1) * 128],
                                     xt[ci][b][:, off:off + 512],
                                     start=(ci == 0), stop=(ci == 1))
            if has_bk:
                for co in range(2):
                    nc.scalar.activation(kb[ib][:, co, :], kps[:, co, :], Id,
                                         bias=bk[:, co:co + 1])
            else:
                # K evacuation on ACT; V evacuation below on DVE (balance)
                nc.scalar.copy(kb[ib][:], kps[:])
            vps = pj.tile([128, 2, NQ], F32, tag="t")
            vps4 = vps[:].rearrange("p a o -> p (a o)").rearrange(
                "p (w c) -> p w c", c=D)
            for w in range(4):
                icol = slice(off + w * 128, off + w * 128 + 128)
                for ci in range(2):
                    nc.tensor.matmul(vps4[:, w, :],
                                     xt[ci][b][:, icol],
                                     wvt[:, ci, :],
                                     start=(ci == 0), stop=(ci == 1))
            vdst = vtb[ib][:, :, :, 0:64]
            vsrc = vps4.rearrange("p w (h c) -> p w h c", h=H)
            nc.vector.tensor_copy(vdst, vsrc)

        # all projections upfront: keeps the PE dense early (HAM warm-up)
        # and the attention loop free of PSUM-pool contention
        for ib in range(8):
            proj_block(ib)

        # projection PSUM pool closes; loop pools take its banks
        pj_ctx.close()
        ps = ctx.enter_context(tc.tile_pool(name="ps", bufs=2, space="PSUM"))
        ps_m = ctx.enter_context(tc.tile_pool(name="psm", bufs=1, space="PSUM"))

        # message-MLP accumulators (live across the whole loop)
        mps = [ps_m.tile([65, NQ], F32, name=f"mps{h}") for h in range(H)]

        def emit_scores_mults(it, el, spt_t):
            # scores + mask-mult for iteration it; each score tile's DVE
            # consumer is emitted before the pool hands its buffer out again
            sbase = spt_t[:, it % 2, :]
            spt_b = bass.AP(tensor=sbase.tensor, offset=sbase.offset,
                            ap=[list(sbase.ap[0]), [0, 2], list(sbase.ap[1])])
            tiles = []
            for hp in range(2):
                sps = ps.tile([128, 2, NQ], F32, tag="t")
                for j in range(2):
                    ro = j * 64
                    nc.tensor.matmul(
                        sps[:, j, :],
                        kb[it // 4][ro:ro + 64, hp,
                                    (it % 4) * 128:(it % 4) * 128 + 128],
                        q_sb[ro:ro + 64, hp, :],
                        start=True, stop=True)
                tiles.append(sps)
            for hp in range(2):
                nc.vector.tensor_mul(el[:, 2 * hp:2 * hp + 2, :],
                                     tiles[hp][:], spt_b)

        def emit_msgs_it(it, e2, last=False):
            # message matmuls for one iteration (4 heads); on the last
            # iteration the per-head denominator gather chases each head's
            # final matmul
            for h in range(H):
                nc.tensor.matmul(mps[h][:], vtb[it // 4][:, it % 4, h, 0:65],
                                 e2[:, h, :],
                                 start=(it == 0), stop=(it == NIT - 1))
                if last:
                    # engine APs must start at a quadrant-aligned partition
                    nc.scalar.copy(dh4[32 * h:32 * h + 1, :], mps[h][64:65, :])

        # ---- main loop: per-iteration scores -> mult -> exp, messages TWO
        # iterations behind (their exp is long done, so the PE never
        # head-of-line blocks on the DVE->ACT chain) ----
        dh4 = sb.tile([128, NQ], F32, name="dh4")
        nc.gpsimd.memset(dh4[:], 1.0)
        e2s = {}
        for it in range(NIT):
            if it % 2 == 0:
                p = it // 2
                if p + 3 < NPAIR:
                    load_spt(p + 3)
                if p == 4:
                    load_late_inputs()
                spt_t = spt_tiles[p]
            if it - 2 in e2s:
                emit_msgs_it(it - 2, e2s.pop(it - 2))
            el = el_pool.tile([128, 4, NQ], BF16, tag="el")
            emit_scores_mults(it, el, spt_t)
            if it % 2 == 1:
                spt_tiles.pop(it // 2)
            e2 = e2_pool.tile([128, 4, NQ], BF16, tag="e2")
            nc.scalar.activation(e2[:], el[:], Exp)
            e2s[it] = e2
        emit_msgs_it(NIT - 2, e2s.pop(NIT - 2))
        emit_msgs_it(NIT - 1, e2s.pop(NIT - 1), last=True)

        # ---- softmax normalization: one batched reciprocal over all four
        # denominator rows (at partitions 0/32/64/96), then per-head
        # broadcast from partition 0 ----
        scr = sb.tile([128, NQ], F32, name="scr")
        rb4 = sb.tile([128, NQ], F32, name="rb4")
        nc.vector.reciprocal_approx_accurate(out=rb4[:], in_=dh4[:],
                                             scratch=scr[:])
        for h in range(H):
            co, ro = h // 2, (h % 2) * 64
            rbh = sb.tile([1, NQ], F32, name=f"rbh{h}")
            nc.scalar.copy(rbh[:], rb4[32 * h:32 * h + 1, :])
            dbc = sb.tile([64, NQ], F32, name=f"dbc{h}")
            nc.gpsimd.partition_broadcast(dbc[:], rbh[:], channels=64)
            nc.vector.tensor_mul(msg[co][ro:ro + 64, :], mps[h][0:64, :],
                                 dbc[:])
            if has_bv:
                nc.scalar.activation(msg[co][ro:ro + 64, :],
                                     msg[co][ro:ro + 64, :], Id,
                                     bias=bv[ro:ro + 64, co:co + 1])

        # ---- message MLP + residual ----
        u1 = ps.tile([128, 2, NQ], F32, tag="t")
        for ci in range(2):
            nc.tensor.matmul(u1[:, 0, :], w1t[:, ci, :], msg[ci][:],
                             start=(ci == 0), stop=(ci == 1))
        h1 = sb.tile([128, NQ], BF16, name="h1")
        if has_b12:
            nc.scalar.activation(h1[:], u1[:, 0, :], Relu, bias=b1[:, 0:1])
        else:
            nc.scalar.activation(h1[:], u1[:, 0, :], Relu)
        u2 = ps.tile([128, 2, NQ], F32, tag="t")
        nc.tensor.matmul(u2[:, 0, :], w2t[:], h1[:], start=True, stop=True)
        h2 = sb.tile([128, NQ], BF16, name="h2")
        if has_b12:
            nc.scalar.activation(h2[:], u2[:, 0, :], Relu, bias=b2[:, 0:1])
        else:
            nc.scalar.activation(h2[:], u2[:, 0, :], Relu)
        for co in range(2):
            u3 = ps.tile([128, 2, NQ], F32, tag="t")
            nc.tensor.matmul(u3[:, 0, :], w3t[:, co * 128:(co + 1) * 128],
                             h2[:], start=True, stop=True)
            ot = sb.tile([128, NQ], F32, name=f"ot{co}")
            if has_b3:
                tb = sb.tile([128, NQ], F32, name=f"tb{co}")
                nc.scalar.activation(tb[:], u3[:, 0, :], Id,
                                     bias=b3[:, co:co + 1])
                nc.vector.tensor_add(ot[:], tb[:], xqr[:, co, :])
            else:
                nc.vector.tensor_add(ot[:], u3[:, 0, :], xqr[:, co, :])
            nc.sync.dma_start(out_d[co * 128:(co + 1) * 128, :], ot[:])

    nc.compile()
    return nc


def _prep_inputs(inputs):
    import ml_dtypes
    bf = lambda a: np.ascontiguousarray(
        np.asarray(a, dtype=np.float32).astype(ml_dtypes.bfloat16))
    f = lambda a: np.ascontiguousarray(np.asarray(a, dtype=np.float32))
    x32 = f(inputs["corr_feat_belief"][0])                  # [D, N]
    spT = np.asarray(inputs["spatial_compatibility"][0]).T  # [N(keys), N(queries)]
    Wq, bq = f(inputs["Wq"]), f(inputs["bq"])
    Wk, bk = f(inputs["Wk"]), f(inputs["bk"])
    Wv, bv = f(inputs["Wv"]), f(inputs["bv"])
    W1, b1, g1, be1 = f(inputs["W1"]), f(inputs["b1"]), f(inputs["g1"]), f(inputs["be1"])
    W2, b2, g2, be2 = f(inputs["W2"]), f(inputs["b2"]), f(inputs["g2"]), f(inputs["be2"])
    W3, b3 = f(inputs["W3"]), f(inputs["b3"])

    scale = np.float32(1.0 / np.sqrt(DH))
    s1 = (g1 / np.sqrt(np.float32(1.0) + np.float32(1e-5))).astype(np.float32)
    s2 = (g2 / np.sqrt(np.float32(1.0) + np.float32(1e-5))).astype(np.float32)

    spT_bf = bf(spT)
    x_bf = bf(x32)
    b1f = (s1 * b1 + be1).astype(np.float32)
    b2f = (s2 * b2 + be2).astype(np.float32)
    common = dict(
        x=x_bf,
        wqt=bf(Wq.T * scale),
        wkt=bf(Wk.T),
        wvt=bf(Wv.T),
        w1t=bf((W1 * s1[:, None]).T),
        w2t=bf((W2 * s2[:, None]).T),
        w3t=bf(W3.T),
        bq2=f((bq * scale).reshape(2, 128).T),
        bk2=f(bk.reshape(2, 128).T),
        bv2=f(bv.reshape(2, 128).T),
        b1f=f(b1f.reshape(128, 1)),
        b2f=f(b2f.reshape(128, 1)),
        b32=f(b3.reshape(2, 128).T),
    )
    in_maps = []
    for m in range(NCORES):
        sl = slice(m * NQ, (m + 1) * NQ)
        im = dict(common)
        im["xq"] = np.ascontiguousarray(x_bf[:, sl])
        im["xqr"] = f(x32[:, sl])
        im["spt"] = np.ascontiguousarray(spT_bf[:, sl])
        in_maps.append(im)
    flags = (bool(np.any(bq != 0)), bool(np.any(bk != 0)),
             bool(np.any(bv != 0)), bool(np.any(b3 != 0)),
             bool(np.any(b1f != 0) or np.any(b2f != 0)))
    return in_maps, flags


def _run(inputs, trace=False):
    from concourse.bass_utils import run_bass_kernel_spmd
    in_maps, flags = _prep_inputs(inputs)
    if flags not in _CACHE:
        _CACHE[flags] = _build(*flags)
    nc = _CACHE[flags]
    res = run_bass_kernel_spmd(nc, in_maps, core_ids=list(range(NCORES)),
                               trace=trace)
    out = np.concatenate([res.results[m]["out"] for m in range(NCORES)],
                         axis=1)[None]
    return np.ascontiguousarray(out.astype(np.float32)), res


def kernel(**inputs):
    out, _ = _run(inputs, trace=False)
    return out


# revision 20
# speedup vs baseline: 1.3667x; 1.0075x over previous
"""Bass/Tile TRN2 kernel for a non-local attention block (BaseNonLocalBlock).

Contract: kernel(**inputs) takes the FULL inputs of the nn.Module problem
(B=1, D=256, H=4, N=4096) and returns the FULL output [1, 256, 4096].

Sharding: query columns of the N x N attention are split across the 8
NeuronCores (512 queries per core). K/V projections are computed
redundantly on every core (cheap); each core produces its own output
column slice and the host concatenates.

Per-core algorithm (flash-attention style, scores never hit HBM):
  Q = (Wq/8) @ xq + bq/8              [256, 512]   (1/sqrt(DH) folded in)
  K = Wk @ x + bk                     [256, 4096]
  V_T = x^T @ Wv^T (+ones col/head)   [4096, 4*65] (denominator trick)
  phase 1: project all of K, V_T (PE-dense, overlaps the input DMA ramp)
  phase 2: per key-chunk i (32 x 128 keys), per head-pair:
    S_T[j] = K_h[:, i]^T @ Q_h        [128, 2, 512]  (PSUM, 2 banks)
    E = exp(spatialT[i] * S_T)        one DVE mult + one ACT exp per pair
    msg_h += V_T_aug[i, h]^T @ E[j]   [65, 512]  (PSUM accum; row 64 = denom)
    (message matmuls run one iteration behind so the PE never head-of-line
    blocks on the DVE->ACT chain; spt tiles prefetched on the gpsimd ring)
  msg = msg_h[0:64] / msg_h[64]  (gpsimd partition_broadcast + DVE recip)
  out = xq + W3 @ relu(bn2(W2 @ relu(bn1(W1 @ msg))))   (BN folded into W/b)

Matmul operands are bf16 (fp32/f32r pay a serialized two-pass weight load
on the PE); accumulation stays fp32 in PSUM, and the residual add reads a
separate fp32 copy of x so the dominant term is exact.
"""

import numpy as np
from contextlib import ExitStack

D = 256
N = 4096
NQ = 512          # queries per core
H = 4
DH = 64
NCORES = 8
NIT = N // 128    # 32 key chunks
VTS = 68          # padded per-head stride in the V_T-aug tile

_CACHE = {}


def _build(has_bq, has_bk, has_bv, has_b3):
    import concourse.bass as bass
    import concourse.tile as tile
    from concourse import bacc, mybir

    F32 = mybir.dt.float32
    BF16 = mybir.dt.bfloat16
    Id = mybir.ActivationFunctionType.Identity
    Exp = mybir.ActivationFunctionType.Exp
    Relu = mybir.ActivationFunctionType.Relu

    nc = bacc.Bacc("TRN2", target_bir_lowering=False, debug=False,
                   num_devices=NCORES)

    # DRAM I/O (per core)
    x_d = nc.dram_tensor("x", [D, N], BF16, kind="ExternalInput").ap()
    xq_d = nc.dram_tensor("xq", [D, NQ], BF16, kind="ExternalInput").ap()
    xqr_d = nc.dram_tensor("xqr", [D, NQ], F32, kind="ExternalInput").ap()
    spt_d = nc.dram_tensor("spt", [N, NQ], BF16, kind="ExternalInput").ap()
    wqt_d = nc.dram_tensor("wqt", [D, D], BF16, kind="ExternalInput").ap()
    wkt_d = nc.dram_tensor("wkt", [D, D], BF16, kind="ExternalInput").ap()
    wvt_d = nc.dram_tensor("wvt", [D, D], BF16, kind="ExternalInput").ap()
    w1t_d = nc.dram_tensor("w1t", [D, 128], BF16, kind="ExternalInput").ap()
    w2t_d = nc.dram_tensor("w2t", [128, 128], BF16, kind="ExternalInput").ap()
    w3t_d = nc.dram_tensor("w3t", [128, D], BF16, kind="ExternalInput").ap()
    bq_d = nc.dram_tensor("bq2", [128, 2], F32, kind="ExternalInput").ap()
    bk_d = nc.dram_tensor("bk2", [128, 2], F32, kind="ExternalInput").ap()
    bv_d = nc.dram_tensor("bv2", [128, 2], F32, kind="ExternalInput").ap()
    b1_d = nc.dram_tensor("b1f", [128, 1], F32, kind="ExternalInput").ap()
    b2_d = nc.dram_tensor("b2f", [128, 1], F32, kind="ExternalInput").ap()
    b3_d = nc.dram_tensor("b32", [128, 2], F32, kind="ExternalInput").ap()
    out_d = nc.dram_tensor("out", [D, NQ], F32, kind="ExternalOutput").ap()

    spt_t3 = spt_d.rearrange("(t p) o -> t p o", p=128)

    with tile.TileContext(nc) as tc, ExitStack() as ctx:
        sb = ctx.enter_context(tc.tile_pool(name="sb", bufs=1))
        spt_pool = ctx.enter_context(tc.tile_pool(name="sptp", bufs=8))
        e_pool = ctx.enter_context(tc.tile_pool(name="ep", bufs=5))
        pj_ctx = ExitStack()
        pj = pj_ctx.enter_context(tc.tile_pool(name="pj", bufs=3, space="PSUM"))

        # ---- ACT exp-table preload (overlaps the DMA ramp) ----
        wtp = sb.tile([1, 2], F32, name="wtp")
        nc.vector.memset(wtp[:], 0.0)
        nc.scalar.activation(wtp[:], wtp[:], Exp)

        # ---- weights + Q inputs first: Q/K/V projections unblock early ----
        wqt = [sb.tile([128, D], BF16, name=f"wqt{ci}") for ci in range(2)]
        wkt = [sb.tile([128, D], BF16, name=f"wkt{ci}") for ci in range(2)]
        wvt = [sb.tile([128, D], BF16, name=f"wvt{ci}") for ci in range(2)]
        # x as 2 (row-chunk) x 8 (column-block) tiles for fine-grained deps
        xcb = [[sb.tile([128, 512], BF16, name=f"x{ci}_{ib}") for ib in range(8)]
               for ci in range(2)]
        xq = [sb.tile([128, NQ], BF16, name=f"xq{co}") for co in range(2)]
        bq = sb.tile([128, 2], F32, name="bq")
        bk = sb.tile([128, 2], F32, name="bk")

        for ci in range(2):
            sl = slice(ci * 128, (ci + 1) * 128)
            nc.sync.dma_start(wkt[ci][:], wkt_d[sl, :])
            nc.sync.dma_start(wvt[ci][:], wvt_d[sl, :])
            nc.sync.dma_start(wqt[ci][:], wqt_d[sl, :])
        for co in range(2):
            nc.sync.dma_start(xq[co][:], xq_d[co * 128:(co + 1) * 128, :])
        nc.sync.dma_start(bq[:], bq_d[:, :])
        nc.sync.dma_start(bk[:], bk_d[:, :])
        for ib in range(8):
            for ci in range(2):
                nc.sync.dma_start(xcb[ci][ib][:],
                                  x_d[ci * 128:(ci + 1) * 128,
                                      ib * 512:(ib + 1) * 512])

        # late inputs issued now (they land mid-loop, well before the tail)
        w1t = [sb.tile([128, 128], BF16, name=f"w1t{ci}") for ci in range(2)]
        for ci in range(2):
            nc.sync.dma_start(w1t[ci][:], w1t_d[ci * 128:(ci + 1) * 128, :])
        w2t = sb.tile([128, 128], BF16, name="w2t")
        nc.sync.dma_start(w2t[:], w2t_d[:, :])
        w3t = sb.tile([128, D], BF16, name="w3t")
        nc.sync.dma_start(w3t[:], w3t_d[:, :])
        xqr = [sb.tile([128, NQ], F32, name=f"xqr{co}") for co in range(2)]
        for co in range(2):
            nc.sync.dma_start(xqr[co][:], xqr_d[co * 128:(co + 1) * 128, :])
        b1 = sb.tile([128, 1], F32, name="b1")
        b2 = sb.tile([128, 1], F32, name="b2")
        nc.sync.dma_start(b1[:], b1_d[:, :])
        nc.sync.dma_start(b2[:], b2_d[:, :])
        if has_bv:
            bv = sb.tile([128, 2], F32, name="bv")
            nc.sync.dma_start(bv[:], bv_d[:, :])
        if has_b3:
            b3 = sb.tile([128, 2], F32, name="b3")
            nc.sync.dma_start(b3[:], b3_d[:, :])

        k_sb = [sb.tile([128, N], BF16, name=f"k{co}") for co in range(2)]
        q_sb = [sb.tile([128, NQ], BF16, name=f"q{co}") for co in range(2)]
        # V^T augmented: per key-chunk it, per head h: [64 V cols | ones | pad]
        vt = sb.tile([128, NIT, H, VTS], BF16, name="vt")
        nc.gpsimd.memset(vt[:, :, :, 64:65], 1.0)
        msg = [sb.tile([128, NQ], BF16, name=f"msg{co}") for co in range(2)]

        # ---- PE warmup: ~4us of tiny matmuls so HAM unthrottles during the
        # DMA ramp (dummy operands; result never read) ----
        warm = sb.tile([128, 64], BF16, name="warm")
        nc.vector.memset(warm[:].bitcast(F32)[:, 0:32], 0.0)
        wps = pj.tile([128, 2, NQ], F32, tag="t")
        for r in range(32):
            nc.tensor.matmul(wps[0:64, 0, 0:64], warm[:], warm[:],
                             start=True, stop=True)


        # ---- main streaming loop over key chunks ----
        # spt prefetch on the (otherwise idle) GPSIMD DMA ring
        spt_tiles = {}

        def load_spt(it):
            t = spt_pool.tile([128, NQ], BF16, tag="spt")
            nc.gpsimd.dma_start(t[:], spt_t3[it])
            spt_tiles[it] = t

        for it in range(4):
            load_spt(it)

        # message matmuls run one iteration behind the scores/mask/exp chain
        # so the PE never waits on the DVE->ACT pipeline mid-iteration
        pend = None

        def emit_msg(p, hp):
            pit, e2s = p
            for j in range(2):
                h = 2 * hp + j
                nc.tensor.matmul(mps[h][:], vt[:, pit, h, 0:65],
                                 e2s[hp][:, j, :],
                                 start=(pit == 0), stop=(pit == NIT - 1))

        # ---- projection phase: all K and V^T blocks (overlaps the DMA ramp,
        # keeps the PE dense/warm; leaves the attention loop contention-free)
        cp = [0]
        for co in range(2):
            ps = pj.tile([128, NQ], F32, tag="t")
            for ci in range(2):
                nc.tensor.matmul(ps[:],
                                 wqt[ci][:, co * 128:(co + 1) * 128],
                                 xq[ci][:],
                                 start=(ci == 0), stop=(ci == 1))
            if has_bq:
                nc.scalar.activation(q_sb[co][:], ps[:], Id,
                                     bias=bq[:, co:co + 1])
            else:
                nc.scalar.copy(q_sb[co][:], ps[:])
        for r in range(16):
            nc.tensor.matmul(wps[0:64, 1, 0:64], warm[:], warm[:],
                             start=True, stop=True)
        for ib in range(8):
            for co in range(2):
                ps = pj.tile([128, 2, NQ], F32, tag="t")
                for ci in range(2):
                    nc.tensor.matmul(ps[:, 0, :],
                                     wkt[ci][:, co * 128:(co + 1) * 128],
                                     xcb[ci][ib][:],
                                     start=(ci == 0), stop=(ci == 1))
                ksl = k_sb[co][:, ib * 512:(ib + 1) * 512]
                if has_bk:
                    nc.scalar.activation(ksl, ps[:, 0, :], Id,
                                         bias=bk[:, co:co + 1])
                elif cp[0] % 2 == 0:
                    nc.scalar.copy(ksl, ps[:, 0, :])
                else:
                    nc.vector.tensor_copy(ksl, ps[:, 0, :])
                cp[0] += 1
            for itp in range(ib * 4, ib * 4 + 4, 2):
                vps = pj.tile([128, 2, NQ], F32, tag="t")
                for w in range(2):
                    icol = slice(((itp + w) % 4) * 128,
                                 ((itp + w) % 4) * 128 + 128)
                    for ci in range(2):
                        nc.tensor.matmul(vps[:, w, 0:D],
                                         xcb[ci][ib][:, icol],
                                         wvt[ci][:],
                                         start=(ci == 0), stop=(ci == 1))
                vdst = vt[:, itp:itp + 2, :, 0:64]
                vsrc = vps[:, 0:2, 0:D].rearrange("p w (h c) -> p w h c", h=H)
                if cp[0] % 2 == 0:
                    nc.scalar.copy(vdst, vsrc)
                else:
                    nc.vector.tensor_copy(vdst, vsrc)
                cp[0] += 1

        pj_ctx.close()
        ps_t = ctx.enter_context(tc.tile_pool(name="pst", bufs=2, space="PSUM"))
        ps_m = ctx.enter_context(tc.tile_pool(name="psm", bufs=1, space="PSUM"))
        mps = [ps_m.tile([65, NQ], F32, name=f"mps{h}") for h in range(H)]

        # ---- attention loop: pure scores -> mask-mult -> exp -> message ----
        for it in range(NIT):
            if True:
                if it + 4 < NIT:
                    load_spt(it + 4)
                spt_t = spt_tiles.pop(it)
                # broadcast the mask over the head pair (free-dim 0-stride)
                spt_b = bass.AP(tensor=spt_t.tensor, offset=spt_t.offset,
                                ap=[list(spt_t.ap[0]), [0, 2],
                                    list(spt_t.ap[1])])
                e2s = []
                for hp in range(2):
                    sps = ps_t.tile([128, 2, NQ], F32, tag="t")
                    for j in range(2):
                        ro = j * 64
                        nc.tensor.matmul(
                            sps[:, j, :],
                            k_sb[hp][ro:ro + 64, it * 128:(it + 1) * 128],
                            q_sb[hp][ro:ro + 64, :],
                            start=True, stop=True)
                    el = e_pool.tile([128, 2, NQ], BF16, tag="el")
                    nc.vector.tensor_mul(el[:], sps[:], spt_b)
                    e2 = e_pool.tile([128, 2, NQ], BF16, tag="e")
                    nc.scalar.activation(e2[:], el[:], Exp)
                    e2s.append(e2)
                    if pend is not None:
                        emit_msg(pend, hp)
                pend = (it, e2s)
        dh4 = sb.tile([128, NQ], F32, name="dh4")
        nc.gpsimd.memset(dh4[:], 1.0)
        for hp in range(2):
            emit_msg(pend, hp)
            for j in range(2):
                h = 2 * hp + j
                nc.scalar.copy(dh4[32 * h:32 * h + 1, :], mps[h][64:65, :])

        # ---- softmax normalization: one batched reciprocal over the four
        # denominator rows (partitions 0/32/64/96), per-head broadcast ----
        scr = sb.tile([128, NQ], F32, name="scr")
        rb4 = sb.tile([128, NQ], F32, name="rb4")
        nc.vector.reciprocal_approx_accurate(out=rb4[:], in_=dh4[:],
                                             scratch=scr[:])
        for h in range(H):
            co, ro = h // 2, (h % 2) * 64
            rbh = sb.tile([1, NQ], F32, name=f"rbh{h}")
            nc.scalar.copy(rbh[:], rb4[32 * h:32 * h + 1, :])
            dbc = sb.tile([64, NQ], F32, name=f"dbc{h}")
            nc.gpsimd.partition_broadcast(dbc[:], rbh[:], channels=64)
            nc.vector.tensor_mul(msg[co][ro:ro + 64, :], mps[h][0:64, :], dbc[:])
            if has_bv:
                nc.scalar.activation(msg[co][ro:ro + 64, :],
                                     msg[co][ro:ro + 64, :], Id,
                                     bias=bv[ro:ro + 64, co:co + 1])

        # ---- message MLP + residual ----
        u1 = ps_t.tile([128, 2, NQ], F32, tag="t")
        for ci in range(2):
            nc.tensor.matmul(u1[:, 0, :], w1t[ci][:], msg[ci][:],
                             start=(ci == 0), stop=(ci == 1))
        h1 = sb.tile([128, NQ], BF16, name="h1")
        nc.scalar.activation(h1[:], u1[:, 0, :], Relu, bias=b1[:, 0:1])
        u2 = ps_t.tile([128, 2, NQ], F32, tag="t")
        nc.tensor.matmul(u2[:, 0, :], w2t[:], h1[:], start=True, stop=True)
        h2 = sb.tile([128, NQ], BF16, name="h2")
        nc.scalar.activation(h2[:], u2[:, 0, :], Relu, bias=b2[:, 0:1])
        for co in range(2):
            u3 = ps_t.tile([128, 2, NQ], F32, tag="t")
            nc.tensor.matmul(u3[:, 0, :], w3t[:, co * 128:(co + 1) * 128],
                             h2[:], start=True, stop=True)
            ot = sb.tile([128, NQ], F32, name=f"ot{co}")
            if has_b3:
                tb = sb.tile([128, NQ], F32, name=f"tb{co}")
                nc.scalar.activation(tb[:], u3[:, 0, :], Id, bias=b3[:, co:co + 1])
                nc.vector.tensor_add(ot[:], tb[:], xqr[co][:])
            else:
                nc.vector.tensor_add(ot[:], u3[:, 0, :], xqr[co][:])
            nc.sync.dma_start(out_d[co * 128:(co + 1) * 128, :], ot[:])

    nc.compile()
    return nc


def _prep_inputs(inputs):
    import ml_dtypes
    bf = lambda a: np.ascontiguousarray(
        np.asarray(a, dtype=np.float32).astype(ml_dtypes.bfloat16))
    f = lambda a: np.ascontiguousarray(np.asarray(a, dtype=np.float32))
    x32 = f(inputs["corr_feat_belief"][0])                  # [D, N]
    spT = np.asarray(inputs["spatial_compatibility"][0]).T  # [N(keys), N(queries)]
    Wq, bq = f(inputs["Wq"]), f(inputs["bq"])
    Wk, bk = f(inputs["Wk"]), f(inputs["bk"])
    Wv, bv = f(inputs["Wv"]), f(inputs["bv"])
    W1, b1, g1, be1 = f(inputs["W1"]), f(inputs["b1"]), f(inputs["g1"]), f(inputs["be1"])
    W2, b2, g2, be2 = f(inputs["W2"]), f(inputs["b2"]), f(inputs["g2"]), f(inputs["be2"])
    W3, b3 = f(inputs["W3"]), f(inputs["b3"])

    scale = np.float32(1.0 / np.sqrt(DH))
    s1 = (g1 / np.sqrt(np.float32(1.0) + np.float32(1e-5))).astype(np.float32)
    s2 = (g2 / np.sqrt(np.float32(1.0) + np.float32(1e-5))).astype(np.float32)

    spT_bf = bf(spT)
    x_bf = bf(x32)
    common = dict(
        x=x_bf,
        wqt=bf(Wq.T * scale),
        wkt=bf(Wk.T),
        wvt=bf(Wv.T),
        w1t=bf((W1 * s1[:, None]).T),
        w2t=bf((W2 * s2[:, None]).T),
        w3t=bf(W3.T),
        bq2=f((bq * scale).reshape(2, 128).T),
        bk2=f(bk.reshape(2, 128).T),
        bv2=f(bv.reshape(2, 128).T),
        b1f=f((s1 * b1 + be1).reshape(128, 1)),
        b2f=f((s2 * b2 + be2).reshape(128, 1)),
        b32=f(b3.reshape(2, 128).T),
    )
    in_maps = []
    for m in range(NCORES):
        sl = slice(m * NQ, (m + 1) * NQ)
        im = dict(common)
        im["xq"] = np.ascontiguousarray(x_bf[:, sl])
        im["xqr"] = f(x32[:, sl])
        im["spt"] = np.ascontiguousarray(spT_bf[:, sl])
        in_maps.append(im)
    flags = tuple(bool(np.any(b != 0)) for b in (bq, bk, bv, b3))
    return in_maps, flags


def _run(inputs, trace=False):
    from concourse.bass_utils import run_bass_kernel_spmd
    in_maps, flags = _prep_inputs(inputs)
    if flags not in _CACHE:
        _CACHE[flags] = _build(*flags)
    nc = _CACHE[flags]
    res = run_bass_kernel_spmd(nc, in_maps, core_ids=list(range(NCORES)),
                               trace=trace)
    out = np.concatenate([res.results[m]["out"] for m in range(NCORES)],
                         axis=1)[None]
    return np.ascontiguousarray(out.astype(np.float32)), res


def kernel(**inputs):
    out, _ = _run(inputs, trace=False)
    return out



# revision 22
# speedup vs baseline: 1.3739x; 1.0053x over previous
"""Bass/Tile TRN2 kernel for a non-local attention block (BaseNonLocalBlock).

Contract: kernel(**inputs) takes the FULL inputs of the nn.Module problem
(B=1, D=256, H=4, N=4096) and returns the FULL output [1, 256, 4096].

Sharding: query columns of the N x N attention are split across the 8
NeuronCores (512 queries per core). K/V projections are computed
redundantly on every core (cheap); each core produces its own output
column slice and the host concatenates.

Per-core algorithm (flash-attention style, scores never hit HBM):
  Q = (Wq/8) @ xq + bq/8              [256, 512]   (1/sqrt(DH) folded in)
  K = Wk @ x + bk                     [256, 4096]
  V_T = x^T @ Wv^T (+ones col/head)   [4096, 4*65] (denominator trick)
  phase 1: project all of K, V_T (PE-dense, overlaps the input DMA ramp)
  phase 2: per key-chunk i (32 x 128 keys), per head-pair:
    S_T[j] = K_h[:, i]^T @ Q_h        [128, 2, 512]  (PSUM, 2 banks)
    E = exp(spatialT[i] * S_T)        one DVE mult + one ACT exp per pair
    msg_h += V_T_aug[i, h]^T @ E[j]   [65, 512]  (PSUM accum; row 64 = denom)
    (message matmuls run one iteration behind so the PE never head-of-line
    blocks on the DVE->ACT chain; spt tiles prefetched on the gpsimd ring)
  msg = msg_h[0:64] / msg_h[64]  (gpsimd partition_broadcast + DVE recip)
  out = xq + W3 @ relu(bn2(W2 @ relu(bn1(W1 @ msg))))   (BN folded into W/b)

Matmul operands are bf16 (fp32/f32r pay a serialized two-pass weight load
on the PE); accumulation stays fp32 in PSUM, and the residual add reads a
separate fp32 copy of x so the dominant term is exact.
"""

import numpy as np
from contextlib import ExitStack

D = 256
N = 4096
NQ = 512          # queries per core
H = 4
DH = 64
NCORES = 8
NIT = N // 128    # 32 key chunks
VTS = 68          # padded per-head stride in the V_T-aug tile

_CACHE = {}


def _build(has_bq, has_bk, has_bv, has_b3):
    import concourse.bass as bass
    import concourse.tile as tile
    from concourse import bacc, mybir

    F32 = mybir.dt.float32
    BF16 = mybir.dt.bfloat16
    Id = mybir.ActivationFunctionType.Identity
    Exp = mybir.ActivationFunctionType.Exp
    Relu = mybir.ActivationFunctionType.Relu

    nc = bacc.Bacc("TRN2", target_bir_lowering=False, debug=False,
                   num_devices=NCORES)

    # DRAM I/O (per core)
    x_d = nc.dram_tensor("x", [D, N], BF16, kind="ExternalInput").ap()
    xq_d = nc.dram_tensor("xq", [D, NQ], BF16, kind="ExternalInput").ap()
    xqr_d = nc.dram_tensor("xqr", [D, NQ], F32, kind="ExternalInput").ap()
    spt_d = nc.dram_tensor("spt", [N, NQ], BF16, kind="ExternalInput").ap()
    wqt_d = nc.dram_tensor("wqt", [D, D], BF16, kind="ExternalInput").ap()
    wkt_d = nc.dram_tensor("wkt", [D, D], BF16, kind="ExternalInput").ap()
    wvt_d = nc.dram_tensor("wvt", [D, D], BF16, kind="ExternalInput").ap()
    w1t_d = nc.dram_tensor("w1t", [D, 128], BF16, kind="ExternalInput").ap()
    w2t_d = nc.dram_tensor("w2t", [128, 128], BF16, kind="ExternalInput").ap()
    w3t_d = nc.dram_tensor("w3t", [128, D], BF16, kind="ExternalInput").ap()
    bq_d = nc.dram_tensor("bq2", [128, 2], F32, kind="ExternalInput").ap()
    bk_d = nc.dram_tensor("bk2", [128, 2], F32, kind="ExternalInput").ap()
    bv_d = nc.dram_tensor("bv2", [128, 2], F32, kind="ExternalInput").ap()
    b1_d = nc.dram_tensor("b1f", [128, 1], F32, kind="ExternalInput").ap()
    b2_d = nc.dram_tensor("b2f", [128, 1], F32, kind="ExternalInput").ap()
    b3_d = nc.dram_tensor("b32", [128, 2], F32, kind="ExternalInput").ap()
    out_d = nc.dram_tensor("out", [D, NQ], F32, kind="ExternalOutput").ap()

    spt_t3 = spt_d.rearrange("(t p) o -> t p o", p=128)

    with tile.TileContext(nc) as tc, ExitStack() as ctx:
        sb = ctx.enter_context(tc.tile_pool(name="sb", bufs=1))
        spt_pool = ctx.enter_context(tc.tile_pool(name="sptp", bufs=8))
        e_pool = ctx.enter_context(tc.tile_pool(name="ep", bufs=5))
        pj_ctx = ExitStack()
        pj = pj_ctx.enter_context(tc.tile_pool(name="pj", bufs=3, space="PSUM"))

        # ---- ACT exp-table preload (overlaps the DMA ramp) ----
        wtp = sb.tile([1, 2], F32, name="wtp")
        nc.vector.memset(wtp[:], 0.0)
        nc.scalar.activation(wtp[:], wtp[:], Exp)

        # ---- weights + Q inputs first: Q/K/V projections unblock early ----
        wqt = [sb.tile([128, D], BF16, name=f"wqt{ci}") for ci in range(2)]
        wkt = [sb.tile([128, D], BF16, name=f"wkt{ci}") for ci in range(2)]
        wvt = [sb.tile([128, D], BF16, name=f"wvt{ci}") for ci in range(2)]
        # x as 2 (row-chunk) x 8 (column-block) tiles for fine-grained deps
        xcb = [[sb.tile([128, 512], BF16, name=f"x{ci}_{ib}") for ib in range(8)]
               for ci in range(2)]
        xq = [sb.tile([128, NQ], BF16, name=f"xq{co}") for co in range(2)]
        bq = sb.tile([128, 2], F32, name="bq")
        bk = sb.tile([128, 2], F32, name="bk")

        for ci in range(2):
            sl = slice(ci * 128, (ci + 1) * 128)
            nc.sync.dma_start(wkt[ci][:], wkt_d[sl, :])
            nc.sync.dma_start(wvt[ci][:], wvt_d[sl, :])
            nc.sync.dma_start(wqt[ci][:], wqt_d[sl, :])
        for co in range(2):
            nc.sync.dma_start(xq[co][:], xq_d[co * 128:(co + 1) * 128, :])
        nc.sync.dma_start(bq[:], bq_d[:, :])
        nc.sync.dma_start(bk[:], bk_d[:, :])
        for ib in range(8):
            for ci in range(2):
                nc.sync.dma_start(xcb[ci][ib][:],
                                  x_d[ci * 128:(ci + 1) * 128,
                                      ib * 512:(ib + 1) * 512])

        # late inputs issued now (they land mid-loop, well before the tail)
        w1t = [sb.tile([128, 128], BF16, name=f"w1t{ci}") for ci in range(2)]
        for ci in range(2):
            nc.sync.dma_start(w1t[ci][:], w1t_d[ci * 128:(ci + 1) * 128, :])
        w2t = sb.tile([128, 128], BF16, name="w2t")
        nc.sync.dma_start(w2t[:], w2t_d[:, :])
        w3t = sb.tile([128, D], BF16, name="w3t")
        nc.sync.dma_start(w3t[:], w3t_d[:, :])
        xqr = [sb.tile([128, NQ], F32, name=f"xqr{co}") for co in range(2)]
        for co in range(2):
            nc.sync.dma_start(xqr[co][:], xqr_d[co * 128:(co + 1) * 128, :])
        b1 = sb.tile([128, 1], F32, name="b1")
        b2 = sb.tile([128, 1], F32, name="b2")
        nc.sync.dma_start(b1[:], b1_d[:, :])
        nc.sync.dma_start(b2[:], b2_d[:, :])
        if has_bv:
            bv = sb.tile([128, 2], F32, name="bv")
            nc.sync.dma_start(bv[:], bv_d[:, :])
        if has_b3:
            b3 = sb.tile([128, 2], F32, name="b3")
            nc.sync.dma_start(b3[:], b3_d[:, :])

        k_sb = [sb.tile([128, N], BF16, name=f"k{co}") for co in range(2)]
        q_sb = [sb.tile([128, NQ], BF16, name=f"q{co}") for co in range(2)]
        # V^T augmented: per key-chunk it, per head h: [64 V cols | ones | pad]
        vt = sb.tile([128, NIT, H, VTS], BF16, name="vt")
        nc.gpsimd.memset(vt[:, :, :, 64:65], 1.0)
        msg = [sb.tile([128, NQ], BF16, name=f"msg{co}") for co in range(2)]

        # ---- PE warmup: ~4us of tiny matmuls so HAM unthrottles during the
        # DMA ramp (dummy operands; result never read) ----
        warm = sb.tile([128, 64], BF16, name="warm")
        nc.vector.memset(warm[:].bitcast(F32)[:, 0:32], 0.0)
        wps = pj.tile([128, 2, NQ], F32, tag="t")
        for r in range(32):
            nc.tensor.matmul(wps[0:64, 0, 0:64], warm[:], warm[:],
                             start=True, stop=True)


        # ---- main streaming loop over key chunks ----
        # spt prefetch on the (otherwise idle) GPSIMD DMA ring
        spt_tiles = {}

        def load_spt(it):
            t = spt_pool.tile([128, NQ], BF16, tag="spt")
            nc.gpsimd.dma_start(t[:], spt_t3[it])
            spt_tiles[it] = t

        for it in range(4):
            load_spt(it)

        # message matmuls run one iteration behind the scores/mask/exp chain
        # so the PE never waits on the DVE->ACT pipeline mid-iteration
        pend = None

        def emit_msg(p, hp):
            pit, e2s = p
            for j in range(2):
                h = 2 * hp + j
                nc.tensor.matmul(mps[h][:], vt[:, pit, h, 0:65],
                                 e2s[hp][:, j, :],
                                 start=(pit == 0), stop=(pit == NIT - 1))

        # ---- projection phase: all K and V^T blocks (overlaps the DMA ramp,
        # keeps the PE dense/warm; leaves the attention loop contention-free)
        cp = [0]
        for co in range(2):
            ps = pj.tile([128, NQ], F32, tag="t")
            for ci in range(2):
                nc.tensor.matmul(ps[:],
                                 wqt[ci][:, co * 128:(co + 1) * 128],
                                 xq[ci][:],
                                 start=(ci == 0), stop=(ci == 1))
            if has_bq:
                nc.scalar.activation(q_sb[co][:], ps[:], Id,
                                     bias=bq[:, co:co + 1])
            else:
                nc.scalar.copy(q_sb[co][:], ps[:])
        for r in range(16):
            nc.tensor.matmul(wps[0:64, 1, 0:64], warm[:], warm[:],
                             start=True, stop=True)
        for ib in range(8):
            for co in range(2):
                ps = pj.tile([128, 2, NQ], F32, tag="t")
                for ci in range(2):
                    nc.tensor.matmul(ps[:, 0, :],
                                     wkt[ci][:, co * 128:(co + 1) * 128],
                                     xcb[ci][ib][:],
                                     start=(ci == 0), stop=(ci == 1))
                ksl = k_sb[co][:, ib * 512:(ib + 1) * 512]
                if has_bk:
                    nc.scalar.activation(ksl, ps[:, 0, :], Id,
                                         bias=bk[:, co:co + 1])
                elif cp[0] % 2 == 0:
                    nc.scalar.copy(ksl, ps[:, 0, :])
                else:
                    nc.vector.tensor_copy(ksl, ps[:, 0, :])
                cp[0] += 1
            for itp in range(ib * 4, ib * 4 + 4, 2):
                vps = pj.tile([128, 2, NQ], F32, tag="t")
                for w in range(2):
                    icol = slice(((itp + w) % 4) * 128,
                                 ((itp + w) % 4) * 128 + 128)
                    for ci in range(2):
                        nc.tensor.matmul(vps[:, w, 0:D],
                                         xcb[ci][ib][:, icol],
                                         wvt[ci][:],
                                         start=(ci == 0), stop=(ci == 1))
                vdst = vt[:, itp:itp + 2, :, 0:64]
                vsrc = vps[:, 0:2, 0:D].rearrange("p w (h c) -> p w h c", h=H)
                if cp[0] % 2 == 0:
                    nc.scalar.copy(vdst, vsrc)
                else:
                    nc.vector.tensor_copy(vdst, vsrc)
                cp[0] += 1

        pj_ctx.close()
        ps_t = ctx.enter_context(tc.tile_pool(name="pst", bufs=2, space="PSUM"))
        ps_m = ctx.enter_context(tc.tile_pool(name="psm", bufs=1, space="PSUM"))
        mps = [ps_m.tile([65, NQ], F32, name=f"mps{h}") for h in range(H)]

        # ---- attention loop: pure scores -> mask-mult -> exp -> message ----
        for it in range(NIT):
            if True:
                if it + 4 < NIT:
                    load_spt(it + 4)
                spt_t = spt_tiles.pop(it)
                # broadcast the mask over the head pair (free-dim 0-stride)
                spt_b = bass.AP(tensor=spt_t.tensor, offset=spt_t.offset,
                                ap=[list(spt_t.ap[0]), [0, 2],
                                    list(spt_t.ap[1])])
                e2s = []
                for hp in range(2):
                    sps = ps_t.tile([128, 2, NQ], F32, tag="t")
                    for j in range(2):
                        ro = j * 64
                        nc.tensor.matmul(
                            sps[:, j, :],
                            k_sb[hp][ro:ro + 64, it * 128:(it + 1) * 128],
                            q_sb[hp][ro:ro + 64, :],
                            start=True, stop=True)
                    el = e_pool.tile([128, 2, NQ], BF16, tag="el")
                    nc.vector.tensor_mul(el[:], sps[:], spt_b)
                    e2 = e_pool.tile([128, 2, NQ], BF16, tag="e")
                    nc.scalar.activation(e2[:], el[:], Exp)
                    e2s.append(e2)
                    if pend is not None:
                        emit_msg(pend, hp)
                pend = (it, e2s)
        dh4 = sb.tile([128, NQ], F32, name="dh4")
        nc.gpsimd.memset(dh4[:], 1.0)
        for hp in range(2):
            emit_msg(pend, hp)
            for j in range(2):
                h = 2 * hp + j
                # gathers split across ACT/DVE so they run in parallel
                if j == 0:
                    nc.scalar.copy(dh4[32 * h:32 * h + 1, :], mps[h][64:65, :])
                else:
                    nc.vector.tensor_copy(dh4[32 * h:32 * h + 1, :],
                                          mps[h][64:65, :])

        # ---- softmax normalization: one batched reciprocal over the four
        # denominator rows (partitions 0/32/64/96), per-head broadcast ----
        scr = sb.tile([128, NQ], F32, name="scr")
        rb4 = sb.tile([128, NQ], F32, name="rb4")
        nc.vector.reciprocal_approx_accurate(out=rb4[:], in_=dh4[:],
                                             scratch=scr[:])
        for h in range(H):
            co, ro = h // 2, (h % 2) * 64
            rbh = sb.tile([1, NQ], F32, name=f"rbh{h}")
            if h % 2 == 0:
                nc.scalar.copy(rbh[:], rb4[32 * h:32 * h + 1, :])
            else:
                nc.vector.tensor_copy(rbh[:], rb4[32 * h:32 * h + 1, :])
            dbc = sb.tile([64, NQ], F32, name=f"dbc{h}")
            nc.gpsimd.partition_broadcast(dbc[:], rbh[:], channels=64)
            nc.vector.tensor_mul(msg[co][ro:ro + 64, :], mps[h][0:64, :], dbc[:])
            if has_bv:
                nc.scalar.activation(msg[co][ro:ro + 64, :],
                                     msg[co][ro:ro + 64, :], Id,
                                     bias=bv[ro:ro + 64, co:co + 1])

        # ---- message MLP + residual ----
        u1 = ps_t.tile([128, 2, NQ], F32, tag="t")
        for ci in range(2):
            nc.tensor.matmul(u1[:, 0, :], w1t[ci][:], msg[ci][:],
                             start=(ci == 0), stop=(ci == 1))
        h1 = sb.tile([128, NQ], BF16, name="h1")
        nc.scalar.activation(h1[:], u1[:, 0, :], Relu, bias=b1[:, 0:1])
        u2 = ps_t.tile([128, 2, NQ], F32, tag="t")
        nc.tensor.matmul(u2[:, 0, :], w2t[:], h1[:], start=True, stop=True)
        h2 = sb.tile([128, NQ], BF16, name="h2")
        nc.scalar.activation(h2[:], u2[:, 0, :], Relu, bias=b2[:, 0:1])
        for co in range(2):
            u3 = ps_t.tile([128, 2, NQ], F32, tag="t")
            nc.tensor.matmul(u3[:, 0, :], w3t[:, co * 128:(co + 1) * 128],
                             h2[:], start=True, stop=True)
            ot = sb.tile([128, NQ], F32, name=f"ot{co}")
            if has_b3:
                tb = sb.tile([128, NQ], F32, name=f"tb{co}")
                nc.scalar.activation(tb[:], u3[:, 0, :], Id, bias=b3[:, co:co + 1])
                nc.vector.tensor_add(ot[:], tb[:], xqr[co][:])
            else:
                nc.vector.tensor_add(ot[:], u3[:, 0, :], xqr[co][:])
            nc.sync.dma_start(out_d[co * 128:(co + 1) * 128, :], ot[:])

    nc.compile()
    return nc


def _prep_inputs(inputs):
    import ml_dtypes
    bf = lambda a: np.ascontiguousarray(
        np.asarray(a, dtype=np.float32).astype(ml_dtypes.bfloat16))
    f = lambda a: np.ascontiguousarray(np.asarray(a, dtype=np.float32))
    x32 = f(inputs["corr_feat_belief"][0])                  # [D, N]
    spT = np.asarray(inputs["spatial_compatibility"][0]).T  # [N(keys), N(queries)]
    Wq, bq = f(inputs["Wq"]), f(inputs["bq"])
    Wk, bk = f(inputs["Wk"]), f(inputs["bk"])
    Wv, bv = f(inputs["Wv"]), f(inputs["bv"])
    W1, b1, g1, be1 = f(inputs["W1"]), f(inputs["b1"]), f(inputs["g1"]), f(inputs["be1"])
    W2, b2, g2, be2 = f(inputs["W2"]), f(inputs["b2"]), f(inputs["g2"]), f(inputs["be2"])
    W3, b3 = f(inputs["W3"]), f(inputs["b3"])

    scale = np.float32(1.0 / np.sqrt(DH))
    s1 = (g1 / np.sqrt(np.float32(1.0) + np.float32(1e-5))).astype(np.float32)
    s2 = (g2 / np.sqrt(np.float32(1.0) + np.float32(1e-5))).astype(np.float32)

    spT_bf = bf(spT)
    x_bf = bf(x32)
    common = dict(
        x=x_bf,
        wqt=bf(Wq.T * scale),
        wkt=bf(Wk.T),
        wvt=bf(Wv.T),
        w1t=bf((W1 * s1[:, None]).T),
        w2t=bf((W2 * s2[:, None]).T),
        w3t=bf(W3.T),
        bq2=f((bq * scale).reshape(2, 128).T),
        bk2=f(bk.reshape(2, 128).T),
        bv2=f(bv.reshape(2, 128).T),
        b1f=f((s1 * b1 + be1).reshape(128, 1)),
        b2f=f((s2 * b2 + be2).reshape(128, 1)),
        b32=f(b3.reshape(2, 128).T),
    )
    in_maps = []
    for m in range(NCORES):
        sl = slice(m * NQ, (m + 1) * NQ)
        im = dict(common)
        im["xq"] = np.ascontiguousarray(x_bf[:, sl])
        im["xqr"] = f(x32[:, sl])
        im["spt"] = np.ascontiguousarray(spT_bf[:, sl])
        in_maps.append(im)
    flags = tuple(bool(np.any(b != 0)) for b in (bq, bk, bv, b3))
    return in_maps, flags


def _run(inputs, trace=False):
    from concourse.bass_utils import run_bass_kernel_spmd
    in_maps, flags = _prep_inputs(inputs)
    if flags not in _CACHE:
        _CACHE[flags] = _build(*flags)
    nc = _CACHE[flags]
    res = run_bass_kernel_spmd(nc, in_maps, core_ids=list(range(NCORES)),
                               trace=trace)
    out = np.concatenate([res.results[m]["out"] for m in range(NCORES)],
                         axis=1)[None]
    return np.ascontiguousarray(out.astype(np.float32)), res


def kernel(**inputs):
    out, _ = _run(inputs, trace=False)
    return out



# revision 23
# speedup vs baseline: 1.3771x; 1.0023x over previous
"""Bass/Tile TRN2 kernel for a non-local attention block (BaseNonLocalBlock).

Contract: kernel(**inputs) takes the FULL inputs of the nn.Module problem
(B=1, D=256, H=4, N=4096) and returns the FULL output [1, 256, 4096].

Sharding: query columns of the N x N attention are split across the 8
NeuronCores (512 queries per core). K/V projections are computed
redundantly on every core (cheap); each core produces its own output
column slice and the host concatenates.

Per-core algorithm (flash-attention style, scores never hit HBM):
  Q = (Wq/8) @ xq + bq/8              [256, 512]   (1/sqrt(DH) folded in)
  K = Wk @ x + bk                     [256, 4096]
  V_T = x^T @ Wv^T (+ones col/head)   [4096, 4*65] (denominator trick)
  phase 1: project all of K, V_T (PE-dense, overlaps the input DMA ramp)
  phase 2: per key-chunk i (32 x 128 keys), per head-pair:
    S_T[j] = K_h[:, i]^T @ Q_h        [128, 2, 512]  (PSUM, 2 banks)
    E = exp(spatialT[i] * S_T)        one DVE mult + one ACT exp per pair
    msg_h += V_T_aug[i, h]^T @ E[j]   [65, 512]  (PSUM accum; row 64 = denom)
    (message matmuls run one iteration behind so the PE never head-of-line
    blocks on the DVE->ACT chain; spt tiles prefetched on the gpsimd ring)
  msg = msg_h[0:64] / msg_h[64]  (gpsimd partition_broadcast + DVE recip)
  out = xq + W3 @ relu(bn2(W2 @ relu(bn1(W1 @ msg))))   (BN folded into W/b)

Matmul operands are bf16 (fp32/f32r pay a serialized two-pass weight load
on the PE); accumulation stays fp32 in PSUM, and the residual add reads a
separate fp32 copy of x so the dominant term is exact.
"""

import numpy as np
from contextlib import ExitStack

D = 256
N = 4096
NQ = 512          # queries per core
H = 4
DH = 64
NCORES = 8
NIT = N // 128    # 32 key chunks
VTS = 68          # padded per-head stride in the V_T-aug tile

_CACHE = {}


def _build(has_bq, has_bk, has_bv, has_b3):
    import concourse.bass as bass
    import concourse.tile as tile
    from concourse import bacc, mybir

    F32 = mybir.dt.float32
    BF16 = mybir.dt.bfloat16
    Id = mybir.ActivationFunctionType.Identity
    Exp = mybir.ActivationFunctionType.Exp
    Relu = mybir.ActivationFunctionType.Relu

    nc = bacc.Bacc("TRN2", target_bir_lowering=False, debug=False,
                   num_devices=NCORES)

    # DRAM I/O (per core)
    x_d = nc.dram_tensor("x", [D, N], BF16, kind="ExternalInput").ap()
    xq_d = nc.dram_tensor("xq", [D, NQ], BF16, kind="ExternalInput").ap()
    xqr_d = nc.dram_tensor("xqr", [D, NQ], F32, kind="ExternalInput").ap()
    spt_d = nc.dram_tensor("spt", [N, NQ], BF16, kind="ExternalInput").ap()
    wqt_d = nc.dram_tensor("wqt", [D, D], BF16, kind="ExternalInput").ap()
    wkt_d = nc.dram_tensor("wkt", [D, D], BF16, kind="ExternalInput").ap()
    wvt_d = nc.dram_tensor("wvt", [D, D], BF16, kind="ExternalInput").ap()
    w1t_d = nc.dram_tensor("w1t", [D, 128], BF16, kind="ExternalInput").ap()
    w2t_d = nc.dram_tensor("w2t", [128, 128], BF16, kind="ExternalInput").ap()
    w3t_d = nc.dram_tensor("w3t", [128, D], BF16, kind="ExternalInput").ap()
    bq_d = nc.dram_tensor("bq2", [128, 2], F32, kind="ExternalInput").ap()
    bk_d = nc.dram_tensor("bk2", [128, 2], F32, kind="ExternalInput").ap()
    bv_d = nc.dram_tensor("bv2", [128, 2], F32, kind="ExternalInput").ap()
    b1_d = nc.dram_tensor("b1f", [128, 1], F32, kind="ExternalInput").ap()
    b2_d = nc.dram_tensor("b2f", [128, 1], F32, kind="ExternalInput").ap()
    b3_d = nc.dram_tensor("b32", [128, 2], F32, kind="ExternalInput").ap()
    out_d = nc.dram_tensor("out", [D, NQ], F32, kind="ExternalOutput").ap()

    spt_t3 = spt_d.rearrange("(t p) o -> t p o", p=128)

    with tile.TileContext(nc) as tc, ExitStack() as ctx:
        sb = ctx.enter_context(tc.tile_pool(name="sb", bufs=1))
        spt_pool = ctx.enter_context(tc.tile_pool(name="sptp", bufs=8))
        e_pool = ctx.enter_context(tc.tile_pool(name="ep", bufs=5))
        pj_ctx = ExitStack()
        pj = pj_ctx.enter_context(tc.tile_pool(name="pj", bufs=3, space="PSUM"))

        # ---- ACT exp-table preload (overlaps the DMA ramp) ----
        wtp = sb.tile([1, 2], F32, name="wtp")
        nc.vector.memset(wtp[:], 0.0)
        nc.scalar.activation(wtp[:], wtp[:], Exp)

        # ---- weights + Q inputs first: Q/K/V projections unblock early ----
        wqt = [sb.tile([128, D], BF16, name=f"wqt{ci}") for ci in range(2)]
        wkt = [sb.tile([128, D], BF16, name=f"wkt{ci}") for ci in range(2)]
        wvt = [sb.tile([128, D], BF16, name=f"wvt{ci}") for ci in range(2)]
        # x as 2 (row-chunk) x 8 (column-block) tiles for fine-grained deps
        xcb = [[sb.tile([128, 512], BF16, name=f"x{ci}_{ib}") for ib in range(8)]
               for ci in range(2)]
        xq = [sb.tile([128, NQ], BF16, name=f"xq{co}") for co in range(2)]
        bq = sb.tile([128, 2], F32, name="bq")
        bk = sb.tile([128, 2], F32, name="bk")

        for ci in range(2):
            sl = slice(ci * 128, (ci + 1) * 128)
            nc.sync.dma_start(wqt[ci][:], wqt_d[sl, :])
        for co in range(2):
            nc.sync.dma_start(xq[co][:], xq_d[co * 128:(co + 1) * 128, :])
        for ci in range(2):
            nc.sync.dma_start(xcb[ci][0][:],
                              x_d[ci * 128:(ci + 1) * 128, 0:512])
        for ci in range(2):
            sl = slice(ci * 128, (ci + 1) * 128)
            nc.sync.dma_start(wkt[ci][:], wkt_d[sl, :])
            nc.sync.dma_start(wvt[ci][:], wvt_d[sl, :])
        if has_bq:
            nc.sync.dma_start(bq[:], bq_d[:, :])
        if has_bk:
            nc.sync.dma_start(bk[:], bk_d[:, :])
        for ib in range(1, 8):
            for ci in range(2):
                nc.sync.dma_start(xcb[ci][ib][:],
                                  x_d[ci * 128:(ci + 1) * 128,
                                      ib * 512:(ib + 1) * 512])

        # late inputs issued now (they land mid-loop, well before the tail)
        w1t = [sb.tile([128, 128], BF16, name=f"w1t{ci}") for ci in range(2)]
        for ci in range(2):
            nc.sync.dma_start(w1t[ci][:], w1t_d[ci * 128:(ci + 1) * 128, :])
        w2t = sb.tile([128, 128], BF16, name="w2t")
        nc.sync.dma_start(w2t[:], w2t_d[:, :])
        w3t = sb.tile([128, D], BF16, name="w3t")
        nc.sync.dma_start(w3t[:], w3t_d[:, :])
        xqr = [sb.tile([128, NQ], F32, name=f"xqr{co}") for co in range(2)]
        for co in range(2):
            nc.sync.dma_start(xqr[co][:], xqr_d[co * 128:(co + 1) * 128, :])
        b1 = sb.tile([128, 1], F32, name="b1")
        b2 = sb.tile([128, 1], F32, name="b2")
        nc.sync.dma_start(b1[:], b1_d[:, :])
        nc.sync.dma_start(b2[:], b2_d[:, :])
        if has_bv:
            bv = sb.tile([128, 2], F32, name="bv")
            nc.sync.dma_start(bv[:], bv_d[:, :])
        if has_b3:
            b3 = sb.tile([128, 2], F32, name="b3")
            nc.sync.dma_start(b3[:], b3_d[:, :])

        k_sb = [sb.tile([128, N], BF16, name=f"k{co}") for co in range(2)]
        q_sb = [sb.tile([128, NQ], BF16, name=f"q{co}") for co in range(2)]
        # V^T augmented: per key-chunk it, per head h: [64 V cols | ones | pad]
        vt = sb.tile([128, NIT, H, VTS], BF16, name="vt")
        nc.gpsimd.memset(vt[:, :, :, 64:65], 1.0)
        msg = [sb.tile([128, NQ], BF16, name=f"msg{co}") for co in range(2)]

        # ---- PE warmup: ~4us of tiny matmuls so HAM unthrottles during the
        # DMA ramp (dummy operands; result never read) ----
        warm = sb.tile([128, 64], BF16, name="warm")
        nc.vector.memset(warm[:].bitcast(F32)[:, 0:32], 0.0)
        wps = pj.tile([128, 2, NQ], F32, tag="t")
        for r in range(32):
            nc.tensor.matmul(wps[0:64, 0, 0:64], warm[:], warm[:],
                             start=True, stop=True)


        # ---- main streaming loop over key chunks ----
        # spt prefetch on the (otherwise idle) GPSIMD DMA ring
        spt_tiles = {}

        def load_spt(it):
            t = spt_pool.tile([128, NQ], BF16, tag="spt")
            nc.gpsimd.dma_start(t[:], spt_t3[it])
            spt_tiles[it] = t

        for it in range(4):
            load_spt(it)

        # message matmuls run one iteration behind the scores/mask/exp chain
        # so the PE never waits on the DVE->ACT pipeline mid-iteration
        pend = None

        def emit_msg(p, hp):
            pit, e2s = p
            for j in range(2):
                h = 2 * hp + j
                nc.tensor.matmul(mps[h][:], vt[:, pit, h, 0:65],
                                 e2s[hp][:, j, :],
                                 start=(pit == 0), stop=(pit == NIT - 1))

        # ---- projection phase: all K and V^T blocks (overlaps the DMA ramp,
        # keeps the PE dense/warm; leaves the attention loop contention-free)
        cp = [0]
        for co in range(2):
            ps = pj.tile([128, NQ], F32, tag="t")
            for ci in range(2):
                nc.tensor.matmul(ps[:],
                                 wqt[ci][:, co * 128:(co + 1) * 128],
                                 xq[ci][:],
                                 start=(ci == 0), stop=(ci == 1))
            if has_bq:
                nc.scalar.activation(q_sb[co][:], ps[:], Id,
                                     bias=bq[:, co:co + 1])
            else:
                nc.scalar.copy(q_sb[co][:], ps[:])
        for r in range(16):
            nc.tensor.matmul(wps[0:64, 1, 0:64], warm[:], warm[:],
                             start=True, stop=True)
        for ib in range(8):
            for co in range(2):
                ps = pj.tile([128, 2, NQ], F32, tag="t")
                for ci in range(2):
                    nc.tensor.matmul(ps[:, 0, :],
                                     wkt[ci][:, co * 128:(co + 1) * 128],
                                     xcb[ci][ib][:],
                                     start=(ci == 0), stop=(ci == 1))
                ksl = k_sb[co][:, ib * 512:(ib + 1) * 512]
                if has_bk:
                    nc.scalar.activation(ksl, ps[:, 0, :], Id,
                                         bias=bk[:, co:co + 1])
                elif cp[0] % 2 == 0:
                    nc.scalar.copy(ksl, ps[:, 0, :])
                else:
                    nc.vector.tensor_copy(ksl, ps[:, 0, :])
                cp[0] += 1
            for itp in range(ib * 4, ib * 4 + 4, 2):
                vps = pj.tile([128, 2, NQ], F32, tag="t")
                for w in range(2):
                    icol = slice(((itp + w) % 4) * 128,
                                 ((itp + w) % 4) * 128 + 128)
                    for ci in range(2):
                        nc.tensor.matmul(vps[:, w, 0:D],
                                         xcb[ci][ib][:, icol],
                                         wvt[ci][:],
                                         start=(ci == 0), stop=(ci == 1))
                vdst = vt[:, itp:itp + 2, :, 0:64]
                vsrc = vps[:, 0:2, 0:D].rearrange("p w (h c) -> p w h c", h=H)
                if cp[0] % 2 == 0:
                    nc.scalar.copy(vdst, vsrc)
                else:
                    nc.vector.tensor_copy(vdst, vsrc)
                cp[0] += 1

        pj_ctx.close()
        ps_t = ctx.enter_context(tc.tile_pool(name="pst", bufs=2, space="PSUM"))
        ps_m = ctx.enter_context(tc.tile_pool(name="psm", bufs=1, space="PSUM"))
        mps = [ps_m.tile([65, NQ], F32, name=f"mps{h}") for h in range(H)]

        # ---- attention loop: pure scores -> mask-mult -> exp -> message ----
        for it in range(NIT):
            if True:
                if it + 4 < NIT:
                    load_spt(it + 4)
                spt_t = spt_tiles.pop(it)
                # broadcast the mask over the head pair (free-dim 0-stride)
                spt_b = bass.AP(tensor=spt_t.tensor, offset=spt_t.offset,
                                ap=[list(spt_t.ap[0]), [0, 2],
                                    list(spt_t.ap[1])])
                e2s = []
                for hp in range(2):
                    sps = ps_t.tile([128, 2, NQ], F32, tag="t")
                    for j in range(2):
                        ro = j * 64
                        nc.tensor.matmul(
                            sps[:, j, :],
                            k_sb[hp][ro:ro + 64, it * 128:(it + 1) * 128],
                            q_sb[hp][ro:ro + 64, :],
                            start=True, stop=True)
                    el = e_pool.tile([128, 2, NQ], BF16, tag="el")
                    nc.vector.tensor_mul(el[:], sps[:], spt_b)
                    e2 = e_pool.tile([128, 2, NQ], BF16, tag="e")
                    nc.scalar.activation(e2[:], el[:], Exp)
                    e2s.append(e2)
                    if pend is not None:
                        emit_msg(pend, hp)
                pend = (it, e2s)
        dh4 = sb.tile([128, NQ], F32, name="dh4")
        nc.gpsimd.memset(dh4[:], 1.0)
        for hp in range(2):
            emit_msg(pend, hp)
            for j in range(2):
                h = 2 * hp + j
                # gathers split across ACT/DVE so they run in parallel
                if j == 0:
                    nc.scalar.copy(dh4[32 * h:32 * h + 1, :], mps[h][64:65, :])
                else:
                    nc.vector.tensor_copy(dh4[32 * h:32 * h + 1, :],
                                          mps[h][64:65, :])

        # keep the PE's HAM activity window busy through the normalization
        # chain so the MLP matmuls below run at full clock
        kwps = ps_t.tile([128, 2, NQ], F32, tag="t")
        for r in range(14):
            nc.tensor.matmul(kwps[0:64, 0, 0:64], warm[:], warm[:],
                             start=True, stop=True)

        # ---- softmax normalization: one batched reciprocal over the four
        # denominator rows (partitions 0/32/64/96), per-head broadcast ----
        scr = sb.tile([128, NQ], F32, name="scr")
        rb4 = sb.tile([128, NQ], F32, name="rb4")
        nc.vector.reciprocal_approx_accurate(out=rb4[:], in_=dh4[:],
                                             scratch=scr[:])
        for h in range(H):
            co, ro = h // 2, (h % 2) * 64
            rbh = sb.tile([1, NQ], F32, name=f"rbh{h}")
            if h % 2 == 0:
                nc.scalar.copy(rbh[:], rb4[32 * h:32 * h + 1, :])
            else:
                nc.vector.tensor_copy(rbh[:], rb4[32 * h:32 * h + 1, :])
            dbc = sb.tile([64, NQ], F32, name=f"dbc{h}")
            nc.gpsimd.partition_broadcast(dbc[:], rbh[:], channels=64)
            nc.vector.tensor_mul(msg[co][ro:ro + 64, :], mps[h][0:64, :], dbc[:])
            if has_bv:
                nc.scalar.activation(msg[co][ro:ro + 64, :],
                                     msg[co][ro:ro + 64, :], Id,
                                     bias=bv[ro:ro + 64, co:co + 1])

        # ---- message MLP + residual ----
        u1 = ps_t.tile([128, 2, NQ], F32, tag="t")
        for ci in range(2):
            nc.tensor.matmul(u1[:, 0, :], w1t[ci][:], msg[ci][:],
                             start=(ci == 0), stop=(ci == 1))
        h1 = sb.tile([128, NQ], BF16, name="h1")
        nc.scalar.activation(h1[:], u1[:, 0, :], Relu, bias=b1[:, 0:1])
        u2 = ps_t.tile([128, 2, NQ], F32, tag="t")
        nc.tensor.matmul(u2[:, 0, :], w2t[:], h1[:], start=True, stop=True)
        h2 = sb.tile([128, NQ], BF16, name="h2")
        nc.scalar.activation(h2[:], u2[:, 0, :], Relu, bias=b2[:, 0:1])
        for co in range(2):
            u3 = ps_t.tile([128, 2, NQ], F32, tag="t")
            nc.tensor.matmul(u3[:, 0, :], w3t[:, co * 128:(co + 1) * 128],
                             h2[:], start=True, stop=True)
            ot = sb.tile([128, NQ], F32, name=f"ot{co}")
            if has_b3:
                tb = sb.tile([128, NQ], F32, name=f"tb{co}")
                nc.scalar.activation(tb[:], u3[:, 0, :], Id, bias=b3[:, co:co + 1])
                nc.vector.tensor_add(ot[:], tb[:], xqr[co][:])
            else:
                nc.vector.tensor_add(ot[:], u3[:, 0, :], xqr[co][:])
            nc.sync.dma_start(out_d[co * 128:(co + 1) * 128, :], ot[:])

    nc.compile()
    return nc


def _prep_inputs(inputs):
    import ml_dtypes
    bf = lambda a: np.ascontiguousarray(
        np.asarray(a, dtype=np.float32).astype(ml_dtypes.bfloat16))
    f = lambda a: np.ascontiguousarray(np.asarray(a, dtype=np.float32))
    x32 = f(inputs["corr_feat_belief"][0])                  # [D, N]
    spT = np.asarray(inputs["spatial_compatibility"][0]).T  # [N(keys), N(queries)]
    Wq, bq = f(inputs["Wq"]), f(inputs["bq"])
    Wk, bk = f(inputs["Wk"]), f(inputs["bk"])
    Wv, bv = f(inputs["Wv"]), f(inputs["bv"])
    W1, b1, g1, be1 = f(inputs["W1"]), f(inputs["b1"]), f(inputs["g1"]), f(inputs["be1"])
    W2, b2, g2, be2 = f(inputs["W2"]), f(inputs["b2"]), f(inputs["g2"]), f(inputs["be2"])
    W3, b3 = f(inputs["W3"]), f(inputs["b3"])

    scale = np.float32(1.0 / np.sqrt(DH))
    s1 = (g1 / np.sqrt(np.float32(1.0) + np.float32(1e-5))).astype(np.float32)
    s2 = (g2 / np.sqrt(np.float32(1.0) + np.float32(1e-5))).astype(np.float32)

    spT_bf = bf(spT)
    x_bf = bf(x32)
    common = dict(
        x=x_bf,
        wqt=bf(Wq.T * scale),
        wkt=bf(Wk.T),
        wvt=bf(Wv.T),
        w1t=bf((W1 * s1[:, None]).T),
        w2t=bf((W2 * s2[:, None]).T),
        w3t=bf(W3.T),
        bq2=f((bq * scale).reshape(2, 128).T),
        bk2=f(bk.reshape(2, 128).T),
        bv2=f(bv.reshape(2, 128).T),
        b1f=f((s1 * b1 + be1).reshape(128, 1)),
        b2f=f((s2 * b2 + be2).reshape(128, 1)),
        b32=f(b3.reshape(2, 128).T),
    )
    in_maps = []
    for m in range(NCORES):
        sl = slice(m * NQ, (m + 1) * NQ)
        im = dict(common)
        im["xq"] = np.ascontiguousarray(x_bf[:, sl])
        im["xqr"] = f(x32[:, sl])
        im["spt"] = np.ascontiguousarray(spT_bf[:, sl])
        in_maps.append(im)
    flags = tuple(bool(np.any(b != 0)) for b in (bq, bk, bv, b3))
    return in_maps, flags


def _run(inputs, trace=False):
    from concourse.bass_utils import run_bass_kernel_spmd
    in_maps, flags = _prep_inputs(inputs)
    if flags not in _CACHE:
        _CACHE[flags] = _build(*flags)
    nc = _CACHE[flags]
    res = run_bass_kernel_spmd(nc, in_maps, core_ids=list(range(NCORES)),
                               trace=trace)
    out = np.concatenate([res.results[m]["out"] for m in range(NCORES)],
                         axis=1)[None]
    return np.ascontiguousarray(out.astype(np.float32)), res


def kernel(**inputs):
    out, _ = _run(inputs, trace=False)
    return out



# revision 24
# speedup vs baseline: 1.3876x; 1.0076x over previous
"""Bass/Tile TRN2 kernel for a non-local attention block (BaseNonLocalBlock).

Contract: kernel(**inputs) takes the FULL inputs of the nn.Module problem
(B=1, D=256, H=4, N=4096) and returns the FULL output [1, 256, 4096].

Sharding: query columns of the N x N attention are split across the 8
NeuronCores (512 queries per core). K/V projections are computed
redundantly on every core (cheap); each core produces its own output
column slice and the host concatenates.

Per-core algorithm (flash-attention style, scores never hit HBM):
  Q = (Wq/8) @ xq + bq/8              [256, 512]   (1/sqrt(DH) folded in)
  K = Wk @ x + bk                     [256, 4096]
  V_T = x^T @ Wv^T (+ones col/head)   [4096, 4*65] (denominator trick)
  phase 1: project all of K, V_T (PE-dense, overlaps the input DMA ramp)
  phase 2: per key-chunk i (32 x 128 keys), per head-pair:
    S_T[j] = K_h[:, i]^T @ Q_h        [128, 2, 512]  (PSUM, 2 banks)
    E = exp(spatialT[i] * S_T)        one DVE mult + one ACT exp per pair
    msg_h += V_T_aug[i, h]^T @ E[j]   [65, 512]  (PSUM accum; row 64 = denom)
    (message matmuls run one iteration behind so the PE never head-of-line
    blocks on the DVE->ACT chain; spt tiles prefetched on the gpsimd ring)
  msg = msg_h[0:64] / msg_h[64]  (gpsimd partition_broadcast + DVE recip)
  out = xq + W3 @ relu(bn2(W2 @ relu(bn1(W1 @ msg))))   (BN folded into W/b)

Matmul operands are bf16 (fp32/f32r pay a serialized two-pass weight load
on the PE); accumulation stays fp32 in PSUM, and the residual add reads a
separate fp32 copy of x so the dominant term is exact.
"""

import numpy as np
from contextlib import ExitStack

D = 256
N = 4096
NQ = 512          # queries per core
H = 4
DH = 64
NCORES = 8
NIT = N // 128    # 32 key chunks
VTS = 68          # padded per-head stride in the V_T-aug tile

_CACHE = {}


def _build(has_bq, has_bk, has_bv, has_b3):
    import concourse.bass as bass
    import concourse.tile as tile
    from concourse import bacc, mybir

    F32 = mybir.dt.float32
    BF16 = mybir.dt.bfloat16
    Id = mybir.ActivationFunctionType.Identity
    Exp = mybir.ActivationFunctionType.Exp
    Relu = mybir.ActivationFunctionType.Relu

    nc = bacc.Bacc("TRN2", target_bir_lowering=False, debug=False,
                   num_devices=NCORES)

    # DRAM I/O (per core)
    x_d = nc.dram_tensor("x", [D, N], BF16, kind="ExternalInput").ap()
    xqr_d = nc.dram_tensor("xqr", [D, NQ], F32, kind="ExternalInput").ap()
    spt_d = nc.dram_tensor("spt", [N, NQ], BF16, kind="ExternalInput").ap()
    wqt_d = nc.dram_tensor("wqt", [D, D], BF16, kind="ExternalInput").ap()
    wkt_d = nc.dram_tensor("wkt", [D, D], BF16, kind="ExternalInput").ap()
    wvt_d = nc.dram_tensor("wvt", [D, D], BF16, kind="ExternalInput").ap()
    w1t_d = nc.dram_tensor("w1t", [D, 128], BF16, kind="ExternalInput").ap()
    w2t_d = nc.dram_tensor("w2t", [128, 128], BF16, kind="ExternalInput").ap()
    w3t_d = nc.dram_tensor("w3t", [128, D], BF16, kind="ExternalInput").ap()
    bq_d = nc.dram_tensor("bq2", [128, 2], F32, kind="ExternalInput").ap()
    bk_d = nc.dram_tensor("bk2", [128, 2], F32, kind="ExternalInput").ap()
    bv_d = nc.dram_tensor("bv2", [128, 2], F32, kind="ExternalInput").ap()
    b1_d = nc.dram_tensor("b1f", [128, 1], F32, kind="ExternalInput").ap()
    b2_d = nc.dram_tensor("b2f", [128, 1], F32, kind="ExternalInput").ap()
    b3_d = nc.dram_tensor("b32", [128, 2], F32, kind="ExternalInput").ap()
    out_d = nc.dram_tensor("out", [D, NQ], F32, kind="ExternalOutput").ap()

    spt_t3 = spt_d.rearrange("(t p) o -> t p o", p=128)

    with tile.TileContext(nc) as tc, ExitStack() as ctx:
        sb = ctx.enter_context(tc.tile_pool(name="sb", bufs=1))
        spt_pool = ctx.enter_context(tc.tile_pool(name="sptp", bufs=8))
        e_pool = ctx.enter_context(tc.tile_pool(name="ep", bufs=5))
        pj_ctx = ExitStack()
        pj = pj_ctx.enter_context(tc.tile_pool(name="pj", bufs=3, space="PSUM"))

        # ---- ACT exp-table preload (overlaps the DMA ramp) ----
        wtp = sb.tile([1, 2], F32, name="wtp")
        nc.vector.memset(wtp[:], 0.0)
        nc.scalar.activation(wtp[:], wtp[:], Exp)

        # ---- weights + Q inputs first: Q/K/V projections unblock early ----
        wqt = [sb.tile([128, D], BF16, name=f"wqt{ci}") for ci in range(2)]
        wkt = [sb.tile([128, D], BF16, name=f"wkt{ci}") for ci in range(2)]
        wvt = [sb.tile([128, D], BF16, name=f"wvt{ci}") for ci in range(2)]
        # x as 2 (row-chunk) x 8 (column-block) tiles for fine-grained deps
        xcb = [[sb.tile([128, 512], BF16, name=f"x{ci}_{ib}") for ib in range(8)]
               for ci in range(2)]
        bq = sb.tile([128, 2], F32, name="bq")
        bk = sb.tile([128, 2], F32, name="bk")

        for ci in range(2):
            sl = slice(ci * 128, (ci + 1) * 128)
            nc.sync.dma_start(wqt[ci][:], wqt_d[sl, :])
        for ci in range(2):
            nc.sync.dma_start(xcb[ci][0][:],
                              x_d[ci * 128:(ci + 1) * 128, 0:512])
        for ci in range(2):
            sl = slice(ci * 128, (ci + 1) * 128)
            nc.sync.dma_start(wkt[ci][:], wkt_d[sl, :])
            nc.sync.dma_start(wvt[ci][:], wvt_d[sl, :])
        if has_bq:
            nc.sync.dma_start(bq[:], bq_d[:, :])
        if has_bk:
            nc.sync.dma_start(bk[:], bk_d[:, :])
        for ib in range(1, 8):
            for ci in range(2):
                nc.sync.dma_start(xcb[ci][ib][:],
                                  x_d[ci * 128:(ci + 1) * 128,
                                      ib * 512:(ib + 1) * 512])

        # late inputs issued now (they land mid-loop, well before the tail)
        w1t = [sb.tile([128, 128], BF16, name=f"w1t{ci}") for ci in range(2)]
        for ci in range(2):
            nc.sync.dma_start(w1t[ci][:], w1t_d[ci * 128:(ci + 1) * 128, :])
        w2t = sb.tile([128, 128], BF16, name="w2t")
        nc.sync.dma_start(w2t[:], w2t_d[:, :])
        w3t = sb.tile([128, D], BF16, name="w3t")
        nc.sync.dma_start(w3t[:], w3t_d[:, :])
        xqr = [sb.tile([128, NQ], F32, name=f"xqr{co}") for co in range(2)]
        for co in range(2):
            nc.sync.dma_start(xqr[co][:], xqr_d[co * 128:(co + 1) * 128, :])
        b1 = sb.tile([128, 1], F32, name="b1")
        b2 = sb.tile([128, 1], F32, name="b2")
        nc.sync.dma_start(b1[:], b1_d[:, :])
        nc.sync.dma_start(b2[:], b2_d[:, :])
        if has_bv:
            bv = sb.tile([128, 2], F32, name="bv")
            nc.sync.dma_start(bv[:], bv_d[:, :])
        if has_b3:
            b3 = sb.tile([128, 2], F32, name="b3")
            nc.sync.dma_start(b3[:], b3_d[:, :])

        k_sb = [sb.tile([128, N], BF16, name=f"k{co}") for co in range(2)]
        q_sb = [sb.tile([128, NQ], BF16, name=f"q{co}") for co in range(2)]
        # V^T augmented: per key-chunk it, per head h: [64 V cols | ones | pad]
        vt = sb.tile([128, NIT, H, VTS], BF16, name="vt")
        nc.gpsimd.memset(vt[:, :, :, 64:65], 1.0)
        msg = [sb.tile([128, NQ], BF16, name=f"msg{co}") for co in range(2)]

        # ---- PE warmup: ~4us of tiny matmuls so HAM unthrottles during the
        # DMA ramp (dummy operands; result never read) ----
        warm = sb.tile([128, 64], BF16, name="warm")
        nc.vector.memset(warm[:].bitcast(F32)[:, 0:32], 0.0)
        wps = pj.tile([128, 2, NQ], F32, tag="t")
        for r in range(32):
            nc.tensor.matmul(wps[0:64, 0, 0:64], warm[:], warm[:],
                             start=True, stop=True)


        # ---- main streaming loop over key chunks ----
        # spt prefetch on the (otherwise idle) GPSIMD DMA ring
        spt_tiles = {}

        def load_spt(it):
            t = spt_pool.tile([128, NQ], BF16, tag="spt")
            nc.gpsimd.dma_start(t[:], spt_t3[it])
            spt_tiles[it] = t

        for it in range(4):
            load_spt(it)

        # message matmuls run one iteration behind the scores/mask/exp chain
        # so the PE never waits on the DVE->ACT pipeline mid-iteration
        pend = None

        def emit_msg(p, hp):
            pit, e2s = p
            for j in range(2):
                h = 2 * hp + j
                nc.tensor.matmul(mps[h][:], vt[:, pit, h, 0:65],
                                 e2s[hp][:, j, :],
                                 start=(pit == 0), stop=(pit == NIT - 1))

        # ---- projection phase: all K and V^T blocks (overlaps the DMA ramp,
        # keeps the PE dense/warm; leaves the attention loop contention-free)
        cp = [0]
        for co in range(2):
            ps = pj.tile([128, NQ], F32, tag="t")
            for ci in range(2):
                nc.tensor.matmul(ps[:],
                                 wqt[ci][:, co * 128:(co + 1) * 128],
                                 xcb[ci][0][:],
                                 start=(ci == 0), stop=(ci == 1))
            if has_bq:
                nc.scalar.activation(q_sb[co][:], ps[:], Id,
                                     bias=bq[:, co:co + 1])
            else:
                nc.scalar.copy(q_sb[co][:], ps[:])
        for r in range(16):
            nc.tensor.matmul(wps[0:64, 1, 0:64], warm[:], warm[:],
                             start=True, stop=True)
        for ib in range(8):
            for co in range(2):
                ps = pj.tile([128, 2, NQ], F32, tag="t")
                for ci in range(2):
                    nc.tensor.matmul(ps[:, 0, :],
                                     wkt[ci][:, co * 128:(co + 1) * 128],
                                     xcb[ci][ib][:],
                                     start=(ci == 0), stop=(ci == 1))
                ksl = k_sb[co][:, ib * 512:(ib + 1) * 512]
                if has_bk:
                    nc.scalar.activation(ksl, ps[:, 0, :], Id,
                                         bias=bk[:, co:co + 1])
                elif cp[0] % 2 == 0:
                    nc.scalar.copy(ksl, ps[:, 0, :])
                else:
                    nc.vector.tensor_copy(ksl, ps[:, 0, :])
                cp[0] += 1
            for itp in range(ib * 4, ib * 4 + 4, 2):
                vps = pj.tile([128, 2, NQ], F32, tag="t")
                for w in range(2):
                    icol = slice(((itp + w) % 4) * 128,
                                 ((itp + w) % 4) * 128 + 128)
                    for ci in range(2):
                        nc.tensor.matmul(vps[:, w, 0:D],
                                         xcb[ci][ib][:, icol],
                                         wvt[ci][:],
                                         start=(ci == 0), stop=(ci == 1))
                vdst = vt[:, itp:itp + 2, :, 0:64]
                vsrc = vps[:, 0:2, 0:D].rearrange("p w (h c) -> p w h c", h=H)
                if cp[0] % 2 == 0:
                    nc.scalar.copy(vdst, vsrc)
                else:
                    nc.vector.tensor_copy(vdst, vsrc)
                cp[0] += 1

        pj_ctx.close()
        ps_t = ctx.enter_context(tc.tile_pool(name="pst", bufs=2, space="PSUM"))
        ps_m = ctx.enter_context(tc.tile_pool(name="psm", bufs=1, space="PSUM"))
        mps = [ps_m.tile([65, NQ], F32, name=f"mps{h}") for h in range(H)]

        # ---- attention loop: pure scores -> mask-mult -> exp -> message ----
        for it in range(NIT):
            if True:
                if it + 4 < NIT:
                    load_spt(it + 4)
                spt_t = spt_tiles.pop(it)
                # broadcast the mask over the head pair (free-dim 0-stride)
                spt_b = bass.AP(tensor=spt_t.tensor, offset=spt_t.offset,
                                ap=[list(spt_t.ap[0]), [0, 2],
                                    list(spt_t.ap[1])])
                e2s = []
                for hp in range(2):
                    sps = ps_t.tile([128, 2, NQ], F32, tag="t")
                    for j in range(2):
                        ro = j * 64
                        nc.tensor.matmul(
                            sps[:, j, :],
                            k_sb[hp][ro:ro + 64, it * 128:(it + 1) * 128],
                            q_sb[hp][ro:ro + 64, :],
                            start=True, stop=True)
                    el = e_pool.tile([128, 2, NQ], BF16, tag="el")
                    nc.vector.tensor_mul(el[:], sps[:], spt_b)
                    e2 = e_pool.tile([128, 2, NQ], BF16, tag="e")
                    nc.scalar.activation(e2[:], el[:], Exp)
                    e2s.append(e2)
                    if pend is not None:
                        emit_msg(pend, hp)
                pend = (it, e2s)
        dh4 = sb.tile([128, NQ], F32, name="dh4")
        nc.gpsimd.memset(dh4[:], 1.0)
        for hp in range(2):
            emit_msg(pend, hp)
            for j in range(2):
                h = 2 * hp + j
                # gathers split across ACT/DVE so they run in parallel
                if j == 0:
                    nc.scalar.copy(dh4[32 * h:32 * h + 1, :], mps[h][64:65, :])
                else:
                    nc.vector.tensor_copy(dh4[32 * h:32 * h + 1, :],
                                          mps[h][64:65, :])

        # keep the PE's HAM activity window busy through the normalization
        # chain so the MLP matmuls below run at full clock
        kwps = ps_t.tile([128, 2, NQ], F32, tag="t")
        for r in range(48):
            nc.tensor.matmul(kwps[0:64, 0, 0:64], warm[:], warm[:],
                             start=True, stop=True)

        # ---- softmax normalization: one batched reciprocal over the four
        # denominator rows (partitions 0/32/64/96), per-head broadcast ----
        scr = sb.tile([128, NQ], F32, name="scr")
        rb4 = sb.tile([128, NQ], F32, name="rb4")
        nc.vector.reciprocal_approx_accurate(out=rb4[:], in_=dh4[:],
                                             scratch=scr[:])
        for h in range(H):
            co, ro = h // 2, (h % 2) * 64
            rbh = sb.tile([1, NQ], F32, name=f"rbh{h}")
            if h % 2 == 0:
                nc.scalar.copy(rbh[:], rb4[32 * h:32 * h + 1, :])
            else:
                nc.vector.tensor_copy(rbh[:], rb4[32 * h:32 * h + 1, :])
            dbc = sb.tile([64, NQ], F32, name=f"dbc{h}")
            nc.gpsimd.partition_broadcast(dbc[:], rbh[:], channels=64)
            nc.vector.tensor_mul(msg[co][ro:ro + 64, :], mps[h][0:64, :], dbc[:])
            if has_bv:
                nc.scalar.activation(msg[co][ro:ro + 64, :],
                                     msg[co][ro:ro + 64, :], Id,
                                     bias=bv[ro:ro + 64, co:co + 1])

        # ---- message MLP + residual ----
        u1 = ps_t.tile([128, 2, NQ], F32, tag="t")
        for ci in range(2):
            nc.tensor.matmul(u1[:, 0, :], w1t[ci][:], msg[ci][:],
                             start=(ci == 0), stop=(ci == 1))
        h1 = sb.tile([128, NQ], BF16, name="h1")
        nc.scalar.activation(h1[:], u1[:, 0, :], Relu, bias=b1[:, 0:1])
        u2 = ps_t.tile([128, 2, NQ], F32, tag="t")
        nc.tensor.matmul(u2[:, 0, :], w2t[:], h1[:], start=True, stop=True)
        h2 = sb.tile([128, NQ], BF16, name="h2")
        nc.scalar.activation(h2[:], u2[:, 0, :], Relu, bias=b2[:, 0:1])
        for co in range(2):
            u3 = ps_t.tile([128, 2, NQ], F32, tag="t")
            nc.tensor.matmul(u3[:, 0, :], w3t[:, co * 128:(co + 1) * 128],
                             h2[:], start=True, stop=True)
            ot = sb.tile([128, NQ], F32, name=f"ot{co}")
            if has_b3:
                tb = sb.tile([128, NQ], F32, name=f"tb{co}")
                nc.scalar.activation(tb[:], u3[:, 0, :], Id, bias=b3[:, co:co + 1])
                nc.vector.tensor_add(ot[:], tb[:], xqr[co][:])
            else:
                nc.vector.tensor_add(ot[:], u3[:, 0, :], xqr[co][:])
            nc.sync.dma_start(out_d[co * 128:(co + 1) * 128, :], ot[:])

    nc.compile()
    return nc


def _prep_inputs(inputs):
    import ml_dtypes
    bf = lambda a: np.ascontiguousarray(
        np.asarray(a, dtype=np.float32).astype(ml_dtypes.bfloat16))
    f = lambda a: np.ascontiguousarray(np.asarray(a, dtype=np.float32))
    x32 = f(inputs["corr_feat_belief"][0])                  # [D, N]
    spT = np.asarray(inputs["spatial_compatibility"][0]).T  # [N(keys), N(queries)]
    Wq, bq = f(inputs["Wq"]), f(inputs["bq"])
    Wk, bk = f(inputs["Wk"]), f(inputs["bk"])
    Wv, bv = f(inputs["Wv"]), f(inputs["bv"])
    W1, b1, g1, be1 = f(inputs["W1"]), f(inputs["b1"]), f(inputs["g1"]), f(inputs["be1"])
    W2, b2, g2, be2 = f(inputs["W2"]), f(inputs["b2"]), f(inputs["g2"]), f(inputs["be2"])
    W3, b3 = f(inputs["W3"]), f(inputs["b3"])

    scale = np.float32(1.0 / np.sqrt(DH))
    s1 = (g1 / np.sqrt(np.float32(1.0) + np.float32(1e-5))).astype(np.float32)
    s2 = (g2 / np.sqrt(np.float32(1.0) + np.float32(1e-5))).astype(np.float32)

    spT_bf = bf(spT)
    x_bf = bf(x32)
    common = dict(
        wqt=bf(Wq.T * scale),
        wkt=bf(Wk.T),
        wvt=bf(Wv.T),
        w1t=bf((W1 * s1[:, None]).T),
        w2t=bf((W2 * s2[:, None]).T),
        w3t=bf(W3.T),
        bq2=f((bq * scale).reshape(2, 128).T),
        bk2=f(bk.reshape(2, 128).T),
        bv2=f(bv.reshape(2, 128).T),
        b1f=f((s1 * b1 + be1).reshape(128, 1)),
        b2f=f((s2 * b2 + be2).reshape(128, 1)),
        b32=f(b3.reshape(2, 128).T),
    )
    in_maps = []
    for m in range(NCORES):
        sl = slice(m * NQ, (m + 1) * NQ)
        # keys permuted so the core's own query block comes first: softmax
        # over keys is permutation-invariant as long as x (keys axis) and
        # spt (rows) are permuted consistently
        perm = np.r_[m * NQ:(m + 1) * NQ, 0:m * NQ, (m + 1) * NQ:N]
        im = dict(common)
        im["x"] = np.ascontiguousarray(x_bf[:, perm])
        im["xqr"] = f(x32[:, sl])
        im["spt"] = np.ascontiguousarray(spT_bf[perm][:, sl])
        in_maps.append(im)
    flags = tuple(bool(np.any(b != 0)) for b in (bq, bk, bv, b3))
    return in_maps, flags


def _run(inputs, trace=False):
    from concourse.bass_utils import run_bass_kernel_spmd
    in_maps, flags = _prep_inputs(inputs)
    if flags not in _CACHE:
        _CACHE[flags] = _build(*flags)
    nc = _CACHE[flags]
    res = run_bass_kernel_spmd(nc, in_maps, core_ids=list(range(NCORES)),
                               trace=trace)
    out = np.concatenate([res.results[m]["out"] for m in range(NCORES)],
                         axis=1)[None]
    return np.ascontiguousarray(out.astype(np.float32)), res


def kernel(**inputs):
    out, _ = _run(inputs, trace=False)
    return out

